# revision 1
# baseline (speedup 1.0000x reference)
"""Trainium2 Bass kernel for a 12-head MHA layer with relative position bias
and a 0/1 attention mask (B=2, N=2048, C=768, H=12, d=64), sharded over 8
NeuronCores (batch x head-group parallel: core c handles batch c//4 and heads
3*(c%4) .. 3*(c%4)+2).

Device math per core (heads i = 0..2, all in transposed "T" layouts):
  qT = (Wq*s).T^T @ xT         (s = d^-0.5 folded into Wq on host)
  ST[k,q]  = kT.T @ qT         (PSUM, per 128-row k-tile)
  E[k,q]   = exp(ST) * EBT     (ACT exp to bf16 + DVE 2x-mode mult;
                                EBT = (exp(rel_bias)*mask).T in bf16)
  otT[d',q]= sum_k v'[k,d'] E[k,q]   (v' = [v | ones] -> row 64 = softmax denom)
  onrm     = otT[0:64] * (1/denom)   (recip via ACT ln+exp, PE outer-product
                                      broadcast across partitions)
  ytT      = pwT.T @ concat_i(onrm)  (partial projection, summed on host)
"""

import os
import numpy as np
import ml_dtypes

import concourse.bass as bass
import concourse.tile as tile
from concourse.tile import add_dep_helper
from concourse import bacc, mybir
from concourse.alu_op_type import AluOpType
from concourse.bass_utils import run_bass_kernel_spmd

AF = mybir.ActivationFunctionType
DT = mybir.dt
F32R = mybir.dt.float32r

B, N, C, H, D = 2, 2048, 768, 12, 64
HPC = H // 4          # heads per core (8 cores = 2 batches x 4 head-groups)
NCORES = 8
SCALE = float(D) ** -0.5

LAST_RESULTS = None   # BassKernelResults of the most recent kernel() call


def _q_chunks(n):
    """Split n into <=512 free-dim chunks for matmul moving operands."""
    out = []
    o = 0
    while o < n:
        sz = min(512, n - o)
        out.append((o, sz))
        o += sz
    return out


def build_program(n=N, c_in=C, hpc=HPC, d=D, c_out=C):
    """Build the per-core Bass/Tile program. Same program runs on all cores
    (SPMD); per-core data differs via in_maps."""
    nt = n // 128                       # number of 128-row k-tiles
    qch = _q_chunks(n)
    ck = (c_in + 127) // 128            # contraction chunks over c_in
    # wqk columns: [q0|q1], [k0|k1], [q2|pad], [k2|pad] in 128-col m-chunks so
    # that each head's qT and kT live at the same partition base (0 or 64).
    n_qk_chunks = 2 * ((hpc + 1) // 2)  # 4 for hpc=3
    wqk_cols = 128 * n_qk_chunks
    wv_cols = hpc * (d + 2)             # [v_i | ones | pad] per head (even stride for fp32r)
    mo = c_out // 128                   # proj output row chunks

    def pc(kc):
        return min(128, c_in - 128 * kc)

    nc = bacc.Bacc("TRN2", target_bir_lowering=False, debug=False)
    xt = nc.dram_tensor("xt", [c_in, n], DT.bfloat16, kind="ExternalInput").ap()
    wqk = nc.dram_tensor("wqk", [c_in, wqk_cols], DT.bfloat16, kind="ExternalInput").ap()
    wv = nc.dram_tensor("wv", [c_in, wv_cols], DT.bfloat16, kind="ExternalInput").ap()
    eb = nc.dram_tensor("eb", [hpc, n, n], DT.bfloat16, kind="ExternalInput").ap()
    pw = nc.dram_tensor("pw", [hpc * d, c_out], DT.bfloat16, kind="ExternalInput").ap()
    yt = nc.dram_tensor("yt", [c_out, n], DT.float32, kind="ExternalOutput").ap()

    with tile.TileContext(nc) as tc:
        # ---- pools (stack allocator: xts/weights released before attn) ----
        persist = tc.alloc_tile_pool(name="persist", bufs=1)
        qkvout = tc.alloc_tile_pool(name="qkvout", bufs=1)
        loadp = tc.alloc_tile_pool(name="loadp", bufs=1)
        ps_qkv = tc.alloc_tile_pool(name="ps_qkv", bufs=4, space="PSUM")

        pw_s = persist.tile([64, hpc, c_out], DT.bfloat16, tag="pw")
        ones_s = persist.tile([1, 128], DT.float32, tag="ones")
        nc.vector.memset(ones_s, 1.0)
        ones3 = persist.tile([128, hpc], DT.float32, tag="ones3")
        nc.vector.memset(ones3, 1.0)
        ones_r = persist.tile([1, 128], F32R, tag="ones_r")
        nc.vector.tensor_copy(ones_r, ones_s)
        for i in range(hpc):
            nc.gpsimd.dma_start(out=pw_s[:, i, :], in_=pw[64 * i:64 * i + 64, :])

        qk_s = qkvout.tile([128, n_qk_chunks, n], DT.bfloat16, tag="qk")
        v_s = qkvout.tile([128, nt, wv_cols], DT.bfloat16, tag="v")

        xts = loadp.tile([128, ck, n], DT.bfloat16, tag="xts")
        wqk_s = loadp.tile([128, ck, wqk_cols], DT.bfloat16, tag="wqk")
        wv_s = loadp.tile([128, ck, wv_cols], DT.bfloat16, tag="wv")
        for kc in range(ck):
            p = pc(kc)
            nc.gpsimd.dma_start(out=xts[:p, kc, :], in_=xt[128 * kc:128 * kc + p, :])
            nc.gpsimd.dma_start(out=wqk_s[:p, kc, :], in_=wqk[128 * kc:128 * kc + p, :])
            nc.gpsimd.dma_start(out=wv_s[:p, kc, :], in_=wv[128 * kc:128 * kc + p, :])

        # ---- phase B: qkT = wqk.T @ xT  -> qk_s ----
        # (m-chunks 0/1 first: they unblock head 0's attention; v' phase C sits
        # between so PE order matches the consumer order)
        def emit_qk_chunk(m):
            for (fo, fs) in qch:
                ps = ps_qkv.tile([128, 512], DT.float32, tag="psqkv", name=f"psB{m}")
                for kc in range(ck):
                    p = pc(kc)
                    nc.tensor.matmul(
                        ps[:, :fs],
                        lhsT=wqk_s[:p, kc, 128 * m:128 * m + 128],
                        rhs=xts[:p, kc, fo:fo + fs],
                        start=(kc == 0), stop=(kc == ck - 1),
                    )
                nc.vector.tensor_copy(qk_s[:, m, fo:fo + fs], ps[:, :fs])

        for m in range(min(2, n_qk_chunks)):
            emit_qk_chunk(m)

        # ---- phase C: v' = xT.T @ wv -> v_s (natural layout, k on partitions)
        for j in range(nt):
            ps = ps_qkv.tile([128, wv_cols], DT.float32, tag="psqkv")
            for kc in range(ck):
                p = pc(kc)
                nc.tensor.matmul(
                    ps,
                    lhsT=xts[:p, kc, 128 * j:128 * j + 128],
                    rhs=wv_s[:p, kc, :],
                    start=(kc == 0), stop=(kc == ck - 1),
                )
            nc.vector.tensor_copy(v_s[:, j, :], ps)
            # ones column for the softmax-denominator trick (fp32r needs a
            # rounding producer, so copy from an fp32 ones scratch)
            nc.vector.tensor_copy(
                v_s[:, j, :].rearrange("p (h c) -> p h c", c=d + 2)[:, :, d],
                ones3)

        for m in range(min(2, n_qk_chunks), n_qk_chunks):
            emit_qk_chunk(m)

        loadp.release()   # free xts/wqk_s/wv_s space for attention pools
        ps_qkv.release()  # free PSUM banks for the attention pools

        # ---- attention pools ----
        ebp = tc.alloc_tile_pool(name="ebp", bufs=4)
        e0p = tc.alloc_tile_pool(name="e0p", bufs=2)
        e1p = tc.alloc_tile_pool(name="e1p", bufs=2)
        normp = tc.alloc_tile_pool(name="normp", bufs=1)
        ps_st = tc.alloc_tile_pool(name="ps_st", bufs=2, space="PSUM")
        ps_ot = tc.alloc_tile_pool(name="ps_ot", bufs=1, space="PSUM")

        osum = [normp.tile([66, n], DT.bfloat16, tag=f"osum{i}", name=f"osum{i}")
                for i in range(hpc)]

        def head_aps(i):
            base = 64 * (i % 2)
            qv = qk_s[base:base + 64, 2 * (i // 2), :]
            kv = qk_s[base:base + 64, 2 * (i // 2) + 1, :]
            return qv, kv

        # ---- phase D: per-head attention ----
        # ST is computed in q-halves with a double-buffered 2-bank PSUM tile so
        # the PE never stalls on the ACT exp drain (keeps HAM at 2.4 GHz).
        hn = min(n, 1024)
        hch = _q_chunks(hn)
        last_exp = None
        for i in range(hpc):
            qv, kv = head_aps(i)
            ot = ps_ot.tile([66, n], DT.float32, tag="ot")
            for j in range(nt):
                eb_t = ebp.tile([128, n], DT.bfloat16, tag="eb")
                nc.sync.dma_start(out=eb_t, in_=eb[i, 128 * j:128 * j + 128, :])
                for h2 in range(n // hn):
                    ho = h2 * hn
                    st = ps_st.tile([128, hn], DT.float32, tag="st")
                    for (fo, fs) in hch:
                        nc.tensor.matmul(
                            st[:, fo:fo + fs],
                            lhsT=kv[:, 128 * j:128 * j + 128],
                            rhs=qv[:, ho + fo:ho + fo + fs],
                            start=True, stop=True,
                        )
                    e0 = e0p.tile([128, hn], DT.bfloat16, tag="e0")
                    last_exp = nc.scalar.activation(e0, st, AF.Exp)
                    e1 = e1p.tile([128, hn], DT.bfloat16, tag="e1")
                    nc.vector.tensor_tensor(e1, e0, eb_t[:, ho:ho + hn],
                                            AluOpType.mult)
                    for (fo, fs) in hch:
                        nc.tensor.matmul(
                            ot[:, ho + fo:ho + fo + fs],
                            lhsT=v_s[:, j, (d + 2) * i:(d + 2) * i + d + 2],
                            rhs=e1[:, fo:fo + fs],
                            start=(j == 0), stop=(j == nt - 1),
                        )
            nc.vector.tensor_copy(osum[i], ot)

        # ---- phase E: normalization (deferred; recip = exp(-ln(sum))) ----
        # Ln's batched before Exp's: keeps ACT table switches to a minimum.
        lnrs = [normp.tile([1, n], DT.float32, tag=f"lnr{i}", name=f"lnr{i}")
                for i in range(hpc)]
        rrows = [normp.tile([1, n], F32R, tag=f"rrow{i}", name=f"rrow{i}")
                 for i in range(hpc)]
        for i in range(hpc):
            ln_inst = nc.scalar.activation(lnrs[i], osum[i][64:65, :], AF.Ln)
            if last_exp is not None:
                # keep all Ln's after the exps: avoids ACT table-set thrash
                add_dep_helper(ln_inst.ins, last_exp.ins, sync=False,
                               reason="act-table ordering")
        for i in range(hpc):
            nc.scalar.activation(rrows[i], lnrs[i], AF.Exp, scale=-1.0)
        for i in range(hpc):
            rrow = rrows[i]
            rsb = normp.tile([64, n], DT.float32, tag="rsb")
            for h2 in range(n // hn):
                ho = h2 * hn
                rps = ps_st.tile([64, hn], DT.float32, tag="st", name="rps")
                for (fo, fs) in hch:
                    nc.tensor.matmul(
                        rps[:, fo:fo + fs],
                        lhsT=ones_r[0:1, 0:64],
                        rhs=rrow[:, ho + fo:ho + fo + fs],
                        start=True, stop=True,
                    )
                nc.vector.tensor_copy(rsb[:, ho:ho + hn], rps)
            # in-place: osum[i][0:64] *= rsb
            nc.vector.tensor_tensor(osum[i][0:64, :], osum[i][0:64, :], rsb,
                                    AluOpType.mult)

        ps_ot.release()
        ps_st.release()

        # ---- phase F: partial projection ytT = pw.T @ onrm ----
        ps_pj = tc.alloc_tile_pool(name="ps_pj", bufs=2, space="PSUM")
        ytp = tc.alloc_tile_pool(name="ytp", bufs=2)
        for m in range(mo):
            ps = ps_pj.tile([128, n], DT.float32, tag="pj")
            for (fo, fs) in qch:
                for i in range(hpc):
                    nc.tensor.matmul(
                        ps[:, fo:fo + fs],
                        lhsT=pw_s[:, i, 128 * m:128 * m + 128],
                        rhs=osum[i][0:64, fo:fo + fs],
                        start=(i == 0), stop=(i == hpc - 1),
                    )
            yts = ytp.tile([128, n], DT.float32, tag="yts")
            nc.vector.tensor_copy(yts, ps)
            nc.sync.dma_start(out=yt[128 * m:128 * m + 128, :], in_=yts)

        ps_pj.release()
        ytp.release()
        normp.release()
        e1p.release()
        e0p.release()
        ebp.release()
        qkvout.release()
        persist.release()

    nc.compile()
    return nc


_PROG = {}


def _get_program(**kw):
    key = tuple(sorted(kw.items()))
    if key not in _PROG:
        _PROG[key] = build_program(**kw)
    return _PROG[key]


def make_in_maps(x, mask, qkv_w, qkv_b, rel_bias, proj_w):
    """Host-side shard + layout prep. Returns list of per-core input dicts."""
    x = np.asarray(x, dtype=np.float32)
    mask = np.asarray(mask)
    qkv_w = np.asarray(qkv_w, dtype=np.float32)
    qkv_b = np.asarray(qkv_b, dtype=np.float32)
    rel_bias = np.asarray(rel_bias, dtype=np.float32)
    proj_w = np.asarray(proj_w, dtype=np.float32)

    n_qk_chunks = 2 * ((HPC + 1) // 2)
    wqk_cols = 128 * n_qk_chunks
    wv_cols = HPC * (D + 2)
    has_bias = bool(np.any(qkv_b))
    c_in = C + 1 if has_bias else C

    # per-batch transposed activations
    xts = []
    for b in range(B):
        xb = x[b].T  # [C, N]
        if has_bias:
            xb = np.concatenate([xb, np.ones((1, N), np.float32)], axis=0)
        xts.append(np.ascontiguousarray(xb))

    maps = []
    for core in range(NCORES):
        b = core // 4
        heads = [HPC * (core % 4) + i for i in range(HPC)]

        wqk = np.zeros((c_in, wqk_cols), np.float32)
        wv = np.zeros((c_in, wv_cols), np.float32)
        pwm = np.zeros((HPC * D, C), np.float32)
        for i, h in enumerate(heads):
            base = 128 * (2 * (i // 2)) + 64 * (i % 2)
            wqk[:C, base:base + 64] = qkv_w[D * h:D * h + D, :].T * SCALE
            kbase = 128 * (2 * (i // 2) + 1) + 64 * (i % 2)
            wqk[:C, kbase:kbase + 64] = qkv_w[C + D * h:C + D * h + D, :].T
            wv[:C, (D + 2) * i:(D + 2) * i + D] = qkv_w[2 * C + D * h:2 * C + D * h + D, :].T
            if has_bias:
                wqk[C, base:base + 64] = qkv_b[D * h:D * h + D] * SCALE
                wqk[C, kbase:kbase + 64] = qkv_b[C + D * h:C + D * h + D]
                wv[C, (D + 2) * i:(D + 2) * i + D] = qkv_b[2 * C + D * h:2 * C + D * h + D]
            pwm[64 * i:64 * i + 64, :] = proj_w[:, D * h:D * h + D].T

        ebs = np.empty((HPC, N, N), ml_dtypes.bfloat16)
        mb = (mask[b, 0] != 0)
        for i, h in enumerate(heads):
            ebs[i] = (np.exp(rel_bias[h]) * mb).T.astype(ml_dtypes.bfloat16)

        maps.append({
            "xt": xts[b].astype(ml_dtypes.bfloat16),
            "wqk": wqk.astype(ml_dtypes.bfloat16),
            "wv": wv.astype(ml_dtypes.bfloat16),
            "eb": ebs,
            "pw": pwm.astype(ml_dtypes.bfloat16),
        })
    return maps, has_bias


def kernel(x, mask, qkv_w, qkv_b, rel_bias, proj_w, proj_b):
    global LAST_RESULTS
    maps, has_bias = make_in_maps(x, mask, qkv_w, qkv_b, rel_bias, proj_w)
    nc = _get_program(c_in=C + 1 if has_bias else C)

    trace = bool(os.environ.get("KERNEL_TRACE"))
    try:
        res = run_bass_kernel_spmd(
            nc, maps, list(range(NCORES)),
            trace=trace,
            trace_cores=list(range(NCORES)) if trace else None,
        )
    except Exception:
        if not trace:
            raise
        # tracing infra unavailable; rerun untraced
        os.environ["BASS_NEVER_TRACE"] = "1"
        res = run_bass_kernel_spmd(nc, maps, list(range(NCORES)), trace=False)
    LAST_RESULTS = res

    proj_b = np.asarray(proj_b, dtype=np.float32)
    out = np.empty((B, N, C), np.float32)
    for b in range(B):
        acc = res.results[4 * b]["yt"].astype(np.float32)
        for c in range(4 * b + 1, 4 * b + 4):
            acc = acc + res.results[c]["yt"]
        out[b] = acc.T + proj_b[None, :]
    return out



# revision 4
# speedup vs baseline: 1.1173x; 1.1173x over previous
"""Trainium2 Bass kernel for a 12-head MHA layer with relative position bias
and a 0/1 attention mask (B=2, N=2048, C=768, H=12, d=64), sharded over 8
NeuronCores (batch x head-group parallel: core c handles batch c//4 and heads
3*(c%4) .. 3*(c%4)+2).

Device math per core (heads i = 0..2, all in transposed "T" layouts):
  qT = (Wq*s).T^T @ xT         (s = d^-0.5 folded into Wq on host)
  ST[k,q]  = kT.T @ qT         (PSUM, per 128-row k-tile)
  E[k,q]   = exp(ST) * EBT     (ACT exp to bf16 + DVE 2x-mode mult;
                                EBT = (exp(rel_bias)*mask).T in bf16)
  otT[d',q]= sum_k v'[k,d'] E[k,q]   (v' = [v | ones] -> row 64 = softmax denom)
  onrm     = otT[0:64] * (1/denom)   (recip on DVE via [16,128] spread, PE
                                      outer-product broadcast across partitions;
                                      interleaved into the next head's loop)
  ytT      = pwT.T @ concat_i(onrm)  (partial projection, summed on host)

The ACT engine runs only exp (one table-set load, warmed at t=0); the
softmax normalization runs entirely on DVE/PE/DMA so the exp stream is
never interrupted.
"""

import os
import numpy as np
import ml_dtypes

import concourse.bass as bass
import concourse.tile as tile
from concourse import bacc, mybir
from concourse.alu_op_type import AluOpType
from concourse.bass_utils import run_bass_kernel_spmd

AF = mybir.ActivationFunctionType
DT = mybir.dt
F32R = mybir.dt.float32r

B, N, C, H, D = 2, 2048, 768, 12, 64
HPC = H // 4          # heads per core (8 cores = 2 batches x 4 head-groups)
NCORES = 8
SCALE = float(D) ** -0.5

LAST_RESULTS = None   # BassKernelResults of the most recent kernel() call


def _q_chunks(n):
    """Split n into <=512 free-dim chunks for matmul moving operands."""
    out = []
    o = 0
    while o < n:
        sz = min(512, n - o)
        out.append((o, sz))
        o += sz
    return out


def build_program(n=N, c_in=C, hpc=HPC, d=D, c_out=C):
    """Build the per-core Bass/Tile program. Same program runs on all cores
    (SPMD); per-core data differs via in_maps."""
    nt = n // 128                       # number of 128-row k-tiles
    qch = _q_chunks(n)
    ck = (c_in + 127) // 128            # contraction chunks over c_in
    # wqk columns: [q0|q1], [k0|k1], [q2|pad], [k2|pad] in 128-col m-chunks so
    # that each head's qT and kT live at the same partition base (0 or 64).
    n_qk_chunks = 2 * ((hpc + 1) // 2)  # 4 for hpc=3
    wqk_cols = 128 * n_qk_chunks
    wv_cols = hpc * (d + 2)             # [v_i | ones | pad] per head (even stride for fp32r)
    mo = c_out // 128                   # proj output row chunks

    def pc(kc):
        return min(128, c_in - 128 * kc)

    nc = bacc.Bacc("TRN2", target_bir_lowering=False, debug=False)
    xt = nc.dram_tensor("xt", [c_in, n], DT.bfloat16, kind="ExternalInput").ap()
    wqk = nc.dram_tensor("wqk", [c_in, wqk_cols], DT.bfloat16, kind="ExternalInput").ap()
    wv = nc.dram_tensor("wv", [c_in, wv_cols], DT.bfloat16, kind="ExternalInput").ap()
    eb = nc.dram_tensor("eb", [hpc, n, n], DT.bfloat16, kind="ExternalInput").ap()
    pw = nc.dram_tensor("pw", [hpc * d, c_out], DT.bfloat16, kind="ExternalInput").ap()
    yt = nc.dram_tensor("yt", [c_out, n], DT.float32, kind="ExternalOutput").ap()

    with tile.TileContext(nc) as tc:
        # ---- pools (stack allocator: xts/weights released before attn) ----
        persist = tc.alloc_tile_pool(name="persist", bufs=1)
        qkvout = tc.alloc_tile_pool(name="qkvout", bufs=1)
        loadp = tc.alloc_tile_pool(name="loadp", bufs=1)
        ps_qkv = tc.alloc_tile_pool(name="ps_qkv", bufs=4, space="PSUM")

        pw_s = persist.tile([64, hpc, c_out], DT.bfloat16, tag="pw")
        ones_s = persist.tile([1, 128], DT.float32, tag="ones")
        nc.vector.memset(ones_s, 1.0)
        ones3 = persist.tile([128, hpc], DT.float32, tag="ones3")
        nc.vector.memset(ones3, 1.0)
        ones_r = persist.tile([1, 128], F32R, tag="ones_r")
        nc.vector.tensor_copy(ones_r, ones_s)
        # warm the ACT exp table-set at t=0 so the first real exp pays no
        # ACT_TABLE_LOAD (~2.7us) on the critical path
        wtmp = persist.tile([1, 128], DT.float32, tag="wtmp")
        nc.scalar.activation(wtmp, ones_s, AF.Exp)
        for i in range(hpc):
            nc.gpsimd.dma_start(out=pw_s[:, i, :], in_=pw[64 * i:64 * i + 64, :])

        qk_s = qkvout.tile([128, n_qk_chunks, n], DT.bfloat16, tag="qk")
        v_s = qkvout.tile([128, nt, wv_cols], DT.bfloat16, tag="v")

        xts = loadp.tile([128, ck, n], DT.bfloat16, tag="xts")
        wqk_s = loadp.tile([128, ck, wqk_cols], DT.bfloat16, tag="wqk")
        wv_s = loadp.tile([128, ck, wv_cols], DT.bfloat16, tag="wv")
        for kc in range(ck):
            p = pc(kc)
            nc.gpsimd.dma_start(out=xts[:p, kc, :], in_=xt[128 * kc:128 * kc + p, :])
            nc.gpsimd.dma_start(out=wqk_s[:p, kc, :], in_=wqk[128 * kc:128 * kc + p, :])
            nc.gpsimd.dma_start(out=wv_s[:p, kc, :], in_=wv[128 * kc:128 * kc + p, :])

        # ---- phase B: qkT = wqk.T @ xT  -> qk_s ----
        # (m-chunks 0/1 first: they unblock head 0's attention; v' phase C sits
        # between so PE order matches the consumer order)
        def emit_qk_chunk(m):
            for (fo, fs) in qch:
                ps = ps_qkv.tile([128, 512], DT.float32, tag="psqkv", name=f"psB{m}")
                for kc in range(ck):
                    p = pc(kc)
                    nc.tensor.matmul(
                        ps[:, :fs],
                        lhsT=wqk_s[:p, kc, 128 * m:128 * m + 128],
                        rhs=xts[:p, kc, fo:fo + fs],
                        start=(kc == 0), stop=(kc == ck - 1),
                    )
                nc.vector.tensor_copy(qk_s[:, m, fo:fo + fs], ps[:, :fs])

        for m in range(min(2, n_qk_chunks)):
            emit_qk_chunk(m)

        # ---- phase C: v' = xT.T @ wv -> v_s (natural layout, k on partitions)
        for j in range(nt):
            ps = ps_qkv.tile([128, wv_cols], DT.float32, tag="psqkv")
            for kc in range(ck):
                p = pc(kc)
                nc.tensor.matmul(
                    ps,
                    lhsT=xts[:p, kc, 128 * j:128 * j + 128],
                    rhs=wv_s[:p, kc, :],
                    start=(kc == 0), stop=(kc == ck - 1),
                )
            nc.vector.tensor_copy(v_s[:, j, :], ps)
            # ones column for the softmax-denominator trick (fp32r needs a
            # rounding producer, so copy from an fp32 ones scratch)
            nc.vector.tensor_copy(
                v_s[:, j, :].rearrange("p (h c) -> p h c", c=d + 2)[:, :, d],
                ones3)

        for m in range(min(2, n_qk_chunks), n_qk_chunks):
            emit_qk_chunk(m)

        loadp.release()   # free xts/wqk_s/wv_s space for attention pools
        ps_qkv.release()  # free PSUM banks for the attention pools

        # ---- attention pools ----
        ebp = tc.alloc_tile_pool(name="ebp", bufs=6)
        e0p = tc.alloc_tile_pool(name="e0p", bufs=4)
        e1p = tc.alloc_tile_pool(name="e1p", bufs=4)
        normp = tc.alloc_tile_pool(name="normp", bufs=1)
        ps_st = tc.alloc_tile_pool(name="ps_st", bufs=2, space="PSUM")
        ps_ot = tc.alloc_tile_pool(name="ps_ot", bufs=1, space="PSUM")

        osum = [normp.tile([64, n], DT.bfloat16, tag=f"osum{i}", name=f"osum{i}")
                for i in range(hpc)]
        dns = [normp.tile([1, n], DT.float32, tag=f"dn{i}", name=f"dn{i}")
               for i in range(hpc)]
        rrows = [normp.tile([1, n], F32R, tag=f"rrow{i}", name=f"rrow{i}")
                 for i in range(hpc)]

        def head_aps(i):
            base = 64 * (i % 2)
            qv = qk_s[base:base + 64, 2 * (i // 2), :]
            kv = qk_s[base:base + 64, 2 * (i // 2) + 1, :]
            return qv, kv

        hn = min(n, 1024)
        hch = _q_chunks(hn)

        # --- softmax-normalization helpers (ACT-free; DVE recip + PE bcast) ---
        def emit_norm_head(i, ot):
            """Emitted right after head i's last OT matmul: stage the f32
            denominator row out of PSUM, spread it 1x2048 -> 16x128, take the
            DVE reciprocal, and gather back to a 1-partition fp32r row for the
            PE broadcast. Returns the rsb tile both bcast halves write into."""
            nc.vector.tensor_copy(dns[i], ot[64:65, :])
            nc.vector.tensor_copy(osum[i], ot[0:64, :])
            dsp = normp.tile([16, 128], DT.float32, tag="dsp", name=f"dsp{i}")
            rsp = normp.tile([16, 128], DT.float32, tag="rsp", name=f"rsp{i}")
            rspr = normp.tile([16, 128], F32R, tag="rspr", name=f"rspr{i}")
            nc.sync.dma_start(out=dsp, in_=dns[i])
            nc.vector.reciprocal(rsp, dsp)
            nc.vector.tensor_copy(rspr, rsp)
            nc.sync.dma_start(out=rrows[i], in_=rspr)
            return normp.tile([64, n], DT.float32, tag="rsb", name=f"rsb{i}")

        def emit_norm_bcast(i, rsb, h2):
            """PE outer-product broadcast of 1/denom across 64 partitions for
            one hn-wide half, folded into osum[i] later via emit_norm_mult."""
            ho = h2 * hn
            rps = ps_st.tile([64, hn], DT.float32, tag="st", name=f"rps{i}_{h2}")
            for (fo, fs) in hch:
                nc.tensor.matmul(
                    rps[:, fo:fo + fs],
                    lhsT=ones_r[0:1, 0:64],
                    rhs=rrows[i][:, ho + fo:ho + fo + fs],
                    start=True, stop=True,
                )
            nc.vector.tensor_copy(rsb[:, ho:ho + hn], rps)

        def emit_norm_mult(i, rsb):
            nc.vector.tensor_tensor(osum[i], osum[i], rsb, AluOpType.mult)

        # ---- phase D: per-head attention, with head i-1's normalization
        # interleaved so the PE/DVE never idle between heads ----
        norm_state = {}

        for i in range(hpc):
            qv, kv = head_aps(i)
            ot = ps_ot.tile([66, n], DT.float32, tag="ot")
            for j in range(nt):
                eb_t = ebp.tile([128, n], DT.bfloat16, tag="eb")
                nc.sync.dma_start(out=eb_t, in_=eb[i, 128 * j:128 * j + 128, :])
                for h2 in range(n // hn):
                    ho = h2 * hn
                    st = ps_st.tile([128, hn], DT.float32, tag="st")
                    for (fo, fs) in hch:
                        nc.tensor.matmul(
                            st[:, fo:fo + fs],
                            lhsT=kv[:, 128 * j:128 * j + 128],
                            rhs=qv[:, ho + fo:ho + fo + fs],
                            start=True, stop=True,
                        )
                    e0 = e0p.tile([128, hn], DT.bfloat16, tag="e0")
                    nc.scalar.activation(e0, st, AF.Exp)
                    e1 = e1p.tile([128, hn], DT.bfloat16, tag="e1")
                    nc.vector.tensor_tensor(e1, e0, eb_t[:, ho:ho + hn],
                                            AluOpType.mult)
                    for (fo, fs) in hch:
                        nc.tensor.matmul(
                            ot[:, ho + fo:ho + fo + fs],
                            lhsT=v_s[:, j, (d + 2) * i:(d + 2) * i + d + 2],
                            rhs=e1[:, fo:fo + fs],
                            start=(j == 0), stop=(j == nt - 1),
                        )
                # interleave previous head's normalization broadcasts/mult
                if i - 1 in norm_state:
                    rsb_prev = norm_state[i - 1]
                    if j == 2:
                        emit_norm_bcast(i - 1, rsb_prev, 0)
                    elif j == 4:
                        emit_norm_bcast(i - 1, rsb_prev, 1)
                    elif j == 6:
                        emit_norm_mult(i - 1, rsb_prev)
                        del norm_state[i - 1]
            norm_state[i] = emit_norm_head(i, ot)

        # tail: normalization of the last head (nothing left to hide it under)
        i = hpc - 1
        rsb_last = norm_state.pop(i)
        for h2 in range(n // hn):
            emit_norm_bcast(i, rsb_last, h2)
        emit_norm_mult(i, rsb_last)

        ps_ot.release()
        ps_st.release()

        # ---- phase F: partial projection ytT = pw.T @ onrm ----
        ps_pj = tc.alloc_tile_pool(name="ps_pj", bufs=2, space="PSUM")
        ytp = tc.alloc_tile_pool(name="ytp", bufs=2)
        for m in range(mo):
            ps = ps_pj.tile([128, n], DT.float32, tag="pj")
            for (fo, fs) in qch:
                for i in range(hpc):
                    nc.tensor.matmul(
                        ps[:, fo:fo + fs],
                        lhsT=pw_s[:, i, 128 * m:128 * m + 128],
                        rhs=osum[i][:, fo:fo + fs],
                        start=(i == 0), stop=(i == hpc - 1),
                    )
            yts = ytp.tile([128, n], DT.float32, tag="yts")
            nc.vector.tensor_copy(yts, ps)
            nc.sync.dma_start(out=yt[128 * m:128 * m + 128, :], in_=yts)

        ps_pj.release()
        ytp.release()
        normp.release()
        e1p.release()
        e0p.release()
        ebp.release()
        qkvout.release()
        persist.release()

    nc.compile()
    return nc


_PROG = {}


def _get_program(**kw):
    key = tuple(sorted(kw.items()))
    if key not in _PROG:
        _PROG[key] = build_program(**kw)
    return _PROG[key]


def make_in_maps(x, mask, qkv_w, qkv_b, rel_bias, proj_w):
    """Host-side shard + layout prep. Returns list of per-core input dicts."""
    x = np.asarray(x, dtype=np.float32)
    mask = np.asarray(mask)
    qkv_w = np.asarray(qkv_w, dtype=np.float32)
    qkv_b = np.asarray(qkv_b, dtype=np.float32)
    rel_bias = np.asarray(rel_bias, dtype=np.float32)
    proj_w = np.asarray(proj_w, dtype=np.float32)

    n_qk_chunks = 2 * ((HPC + 1) // 2)
    wqk_cols = 128 * n_qk_chunks
    wv_cols = HPC * (D + 2)
    has_bias = bool(np.any(qkv_b))
    c_in = C + 1 if has_bias else C

    # per-batch transposed activations
    xts = []
    for b in range(B):
        xb = x[b].T  # [C, N]
        if has_bias:
            xb = np.concatenate([xb, np.ones((1, N), np.float32)], axis=0)
        xts.append(np.ascontiguousarray(xb))

    maps = []
    for core in range(NCORES):
        b = core // 4
        heads = [HPC * (core % 4) + i for i in range(HPC)]

        wqk = np.zeros((c_in, wqk_cols), np.float32)
        wv = np.zeros((c_in, wv_cols), np.float32)
        pwm = np.zeros((HPC * D, C), np.float32)
        for i, h in enumerate(heads):
            base = 128 * (2 * (i // 2)) + 64 * (i % 2)
            wqk[:C, base:base + 64] = qkv_w[D * h:D * h + D, :].T * SCALE
            kbase = 128 * (2 * (i // 2) + 1) + 64 * (i % 2)
            wqk[:C, kbase:kbase + 64] = qkv_w[C + D * h:C + D * h + D, :].T
            wv[:C, (D + 2) * i:(D + 2) * i + D] = qkv_w[2 * C + D * h:2 * C + D * h + D, :].T
            if has_bias:
                wqk[C, base:base + 64] = qkv_b[D * h:D * h + D] * SCALE
                wqk[C, kbase:kbase + 64] = qkv_b[C + D * h:C + D * h + D]
                wv[C, (D + 2) * i:(D + 2) * i + D] = qkv_b[2 * C + D * h:2 * C + D * h + D]
            pwm[64 * i:64 * i + 64, :] = proj_w[:, D * h:D * h + D].T

        ebs = np.empty((HPC, N, N), ml_dtypes.bfloat16)
        mb = (mask[b, 0] != 0)
        for i, h in enumerate(heads):
            ebs[i] = (np.exp(rel_bias[h]) * mb).T.astype(ml_dtypes.bfloat16)

        maps.append({
            "xt": xts[b].astype(ml_dtypes.bfloat16),
            "wqk": wqk.astype(ml_dtypes.bfloat16),
            "wv": wv.astype(ml_dtypes.bfloat16),
            "eb": ebs,
            "pw": pwm.astype(ml_dtypes.bfloat16),
        })
    return maps, has_bias


def kernel(x, mask, qkv_w, qkv_b, rel_bias, proj_w, proj_b):
    global LAST_RESULTS
    maps, has_bias = make_in_maps(x, mask, qkv_w, qkv_b, rel_bias, proj_w)
    nc = _get_program(c_in=C + 1 if has_bias else C)

    trace = bool(os.environ.get("KERNEL_TRACE"))
    try:
        res = run_bass_kernel_spmd(
            nc, maps, list(range(NCORES)),
            trace=trace,
            trace_cores=list(range(NCORES)) if trace else None,
        )
    except Exception:
        if not trace:
            raise
        # tracing infra unavailable; rerun untraced
        os.environ["BASS_NEVER_TRACE"] = "1"
        res = run_bass_kernel_spmd(nc, maps, list(range(NCORES)), trace=False)
    LAST_RESULTS = res

    proj_b = np.asarray(proj_b, dtype=np.float32)
    out = np.empty((B, N, C), np.float32)
    for b in range(B):
        acc = res.results[4 * b]["yt"].astype(np.float32)
        for c in range(4 * b + 1, 4 * b + 4):
            acc = acc + res.results[c]["yt"]
        out[b] = acc.T + proj_b[None, :]
    return out


# revision 9
# speedup vs baseline: 1.1292x; 1.0107x over previous
"""Trainium2 Bass kernel for a 12-head MHA layer with relative position bias
and a 0/1 attention mask (B=2, N=2048, C=768, H=12, d=64), sharded over 8
NeuronCores (batch x head-group parallel: core c handles batch c//4 and heads
3*(c%4) .. 3*(c%4)+2).

Device math per core (heads i = 0..2, all in transposed "T" layouts):
  qT = (Wq*s).T^T @ xT         (s = d^-0.5 folded into Wq on host)
  ST[k,q]  = kT.T @ qT         (PSUM, per 128-row k-tile)
  E[k,q]   = exp(ST) * EBT     (ACT exp to bf16 + DVE 2x-mode mult;
                                EBT = (exp(rel_bias)*mask).T in bf16)
  otT[d',q]= sum_k v'[k,d'] E[k,q]   (v' = [v | ones] -> row 64 = softmax denom)
  onrm     = otT[0:64] * (1/denom)   (recip on DVE via [16,128] spread, PE
                                      outer-product broadcast across partitions;
                                      interleaved into the next head's loop)
  ytT      = pwT.T @ concat_i(onrm)  (partial projection, summed on host)

The ACT engine runs only exp (one table-set load, warmed at t=0); the
softmax normalization runs entirely on DVE/PE/DMA so the exp stream is
never interrupted.
"""

import os
import numpy as np
import ml_dtypes

import concourse.bass as bass
import concourse.tile as tile
from concourse.tile import add_dep_helper
from concourse import bacc, mybir
from concourse.alu_op_type import AluOpType
from concourse.bass_utils import run_bass_kernel_spmd

AF = mybir.ActivationFunctionType
DT = mybir.dt
F32R = mybir.dt.float32r

B, N, C, H, D = 2, 2048, 768, 12, 64
HPC = H // 4          # heads per core (8 cores = 2 batches x 4 head-groups)
NCORES = 8
SCALE = float(D) ** -0.5

LAST_RESULTS = None   # BassKernelResults of the most recent kernel() call


def _q_chunks(n):
    """Split n into <=512 free-dim chunks for matmul moving operands."""
    out = []
    o = 0
    while o < n:
        sz = min(512, n - o)
        out.append((o, sz))
        o += sz
    return out


def build_program(n=N, c_in=C, hpc=HPC, d=D, c_out=C):
    """Build the per-core Bass/Tile program. Same program runs on all cores
    (SPMD); per-core data differs via in_maps."""
    nt = n // 128                       # number of 128-row k-tiles
    qch = _q_chunks(n)
    ck = (c_in + 127) // 128            # contraction chunks over c_in
    # wqk columns: [q0|q1], [k0|k1], [q2|pad], [k2|pad] in 128-col m-chunks so
    # that each head's qT and kT live at the same partition base (0 or 64).
    n_qk_chunks = 2 * ((hpc + 1) // 2)  # 4 for hpc=3
    wqk_cols = 128 * n_qk_chunks
    wv_cols = hpc * (d + 2)             # [v_i | ones | pad] per head (even stride for fp32r)
    mo = c_out // 128                   # proj output row chunks

    def pc(kc):
        return min(128, c_in - 128 * kc)

    nc = bacc.Bacc("TRN2", target_bir_lowering=False, debug=False)
    xt = nc.dram_tensor("xt", [c_in, n], DT.bfloat16, kind="ExternalInput").ap()
    wqk = nc.dram_tensor("wqk", [c_in, wqk_cols], DT.bfloat16, kind="ExternalInput").ap()
    wv = nc.dram_tensor("wv", [c_in, wv_cols], DT.bfloat16, kind="ExternalInput").ap()
    eb = nc.dram_tensor("eb", [hpc, n, n], DT.bfloat16, kind="ExternalInput").ap()
    pw = nc.dram_tensor("pw", [hpc * d, c_out], DT.bfloat16, kind="ExternalInput").ap()
    yt = nc.dram_tensor("yt", [c_out, n], DT.float32, kind="ExternalOutput").ap()

    with tile.TileContext(nc) as tc:
        # ---- pools (stack allocator: xts/weights released before attn) ----
        persist = tc.alloc_tile_pool(name="persist", bufs=1)
        qkvout = tc.alloc_tile_pool(name="qkvout", bufs=1)
        loadp = tc.alloc_tile_pool(name="loadp", bufs=1)
        ps_qkv = tc.alloc_tile_pool(name="ps_qkv", bufs=4, space="PSUM")

        pw_s = persist.tile([64, hpc, c_out], DT.bfloat16, tag="pw")
        ones_s = persist.tile([1, 128], DT.float32, tag="ones")
        nc.vector.memset(ones_s, 1.0)
        ones3 = persist.tile([128, hpc], DT.float32, tag="ones3")
        nc.vector.memset(ones3, 1.0)
        ones_r = persist.tile([1, 128], F32R, tag="ones_r")
        nc.vector.tensor_copy(ones_r, ones_s)
        # warm the ACT exp table-set at t=0 so the first real exp pays no
        # ACT_TABLE_LOAD (~2.7us) on the critical path
        wtmp = persist.tile([1, 128], DT.float32, tag="wtmp")
        nc.scalar.activation(wtmp, ones_s, AF.Exp)
        for i in range(hpc):
            nc.gpsimd.dma_start(out=pw_s[:, i, :], in_=pw[64 * i:64 * i + 64, :])

        qk_s = qkvout.tile([128, n_qk_chunks, n], DT.bfloat16, tag="qk")
        v_s = qkvout.tile([128, nt, wv_cols], DT.bfloat16, tag="v")

        xts = loadp.tile([128, ck, n], DT.bfloat16, tag="xts")
        wqk_s = loadp.tile([128, ck, wqk_cols], DT.bfloat16, tag="wqk")
        wv_s = loadp.tile([128, ck, wv_cols], DT.bfloat16, tag="wv")
        last_in_dma = None
        for kc in range(ck):
            p = pc(kc)
            nc.gpsimd.dma_start(out=xts[:p, kc, :], in_=xt[128 * kc:128 * kc + p, :])
            nc.gpsimd.dma_start(out=wqk_s[:p, kc, :], in_=wqk[128 * kc:128 * kc + p, :])
            last_in_dma = nc.gpsimd.dma_start(
                out=wv_s[:p, kc, :], in_=wv[128 * kc:128 * kc + p, :])

        # ---- phase B: qkT = wqk.T @ xT  -> qk_s ----
        # (m-chunks 0/1 first: they unblock head 0's attention; v' phase C sits
        # between so PE order matches the consumer order)
        def emit_qk_chunk(m):
            for (fo, fs) in qch:
                ps = ps_qkv.tile([128, 512], DT.float32, tag="psqkv", name=f"psB{m}")
                for kc in range(ck):
                    p = pc(kc)
                    nc.tensor.matmul(
                        ps[:, :fs],
                        lhsT=wqk_s[:p, kc, 128 * m:128 * m + 128],
                        rhs=xts[:p, kc, fo:fo + fs],
                        start=(kc == 0), stop=(kc == ck - 1),
                    )
                nc.vector.tensor_copy(qk_s[:, m, fo:fo + fs], ps[:, :fs])

        for m in range(min(2, n_qk_chunks)):
            emit_qk_chunk(m)

        # ---- phase C: v' = xT.T @ wv -> v_s (natural layout, k on partitions)
        for j in range(nt):
            ps = ps_qkv.tile([128, wv_cols], DT.float32, tag="psqkv")
            for kc in range(ck):
                p = pc(kc)
                nc.tensor.matmul(
                    ps,
                    lhsT=xts[:p, kc, 128 * j:128 * j + 128],
                    rhs=wv_s[:p, kc, :],
                    start=(kc == 0), stop=(kc == ck - 1),
                )
            nc.vector.tensor_copy(v_s[:, j, :], ps)
            # ones column for the softmax-denominator trick (fp32r needs a
            # rounding producer, so copy from an fp32 ones scratch)
            nc.vector.tensor_copy(
                v_s[:, j, :].rearrange("p (h c) -> p h c", c=d + 2)[:, :, d],
                ones3)

        for m in range(min(2, n_qk_chunks), n_qk_chunks):
            emit_qk_chunk(m)

        loadp.release()   # free xts/wqk_s/wv_s space for attention pools
        ps_qkv.release()  # free PSUM banks for the attention pools

        # ---- attention pools ----
        ebp = tc.alloc_tile_pool(name="ebp", bufs=6)
        e0p = tc.alloc_tile_pool(name="e0p", bufs=4)
        e1p = tc.alloc_tile_pool(name="e1p", bufs=4)
        normp = tc.alloc_tile_pool(name="normp", bufs=1)
        ps_st = tc.alloc_tile_pool(name="ps_st", bufs=2, space="PSUM")
        ps_ot = tc.alloc_tile_pool(name="ps_ot", bufs=1, space="PSUM")

        osum = [normp.tile([64, n], DT.bfloat16, tag=f"osum{i}", name=f"osum{i}")
                for i in range(hpc)]
        dns = [normp.tile([1, n], DT.float32, tag=f"dn{i}", name=f"dn{i}")
               for i in range(hpc)]
        rrows = [normp.tile([1, n], F32R, tag=f"rrow{i}", name=f"rrow{i}")
                 for i in range(hpc)]

        def head_aps(i):
            base = 64 * (i % 2)
            qv = qk_s[base:base + 64, 2 * (i // 2), :]
            kv = qk_s[base:base + 64, 2 * (i // 2) + 1, :]
            return qv, kv

        hn = min(n, 1024)
        hch = _q_chunks(hn)

        # --- softmax-normalization helpers (ACT-free; DVE recip + PE bcast) ---
        def emit_norm_head(i, ot):
            """Emitted right after head i's last OT matmul: stage the f32
            denominator row out of PSUM, spread it 1x2048 -> 16x128, take the
            DVE reciprocal, and gather back to a 1-partition fp32r row for the
            PE broadcast. Returns the rsb tile both bcast halves write into."""
            nc.vector.tensor_copy(dns[i], ot[64:65, :])
            nc.vector.tensor_copy(osum[i], ot[0:64, :])
            dsp = normp.tile([16, 128], DT.float32, tag="dsp", name=f"dsp{i}")
            rsp = normp.tile([16, 128], DT.float32, tag="rsp", name=f"rsp{i}")
            rspr = normp.tile([16, 128], F32R, tag="rspr", name=f"rspr{i}")
            nc.sync.dma_start(out=dsp, in_=dns[i])
            nc.vector.reciprocal(rsp, dsp)
            nc.vector.tensor_copy(rspr, rsp)
            nc.sync.dma_start(out=rrows[i], in_=rspr)
            return normp.tile([64, n], DT.float32, tag="rsb", name=f"rsb{i}")

        def emit_norm_bcast(i, rsb, h2):
            """PE outer-product broadcast of 1/denom across 64 partitions for
            one hn-wide half, folded into osum[i] later via emit_norm_mult."""
            ho = h2 * hn
            rps = ps_st.tile([64, hn], DT.float32, tag="st", name=f"rps{i}_{h2}")
            for (fo, fs) in hch:
                nc.tensor.matmul(
                    rps[:, fo:fo + fs],
                    lhsT=ones_r[0:1, 0:64],
                    rhs=rrows[i][:, ho + fo:ho + fo + fs],
                    start=True, stop=True,
                )
            nc.vector.tensor_copy(rsb[:, ho:ho + hn], rps)

        def emit_norm_mult(i, rsb):
            nc.vector.tensor_tensor(osum[i], osum[i], rsb, AluOpType.mult)

        # ---- phase D: per-head attention, software-pipelined ----
        # The OT matmuls for unit U are emitted one unit LATE so the in-order
        # PE queue is [S(U+1), OT(U)]: both are dependency-ready when the PE
        # reaches them (S(U+1) only needs exp(U-1)'s PSUM drain; OT(U) only
        # needs mult(U), which ran during exp(U)). Emitting OT(U) right after
        # mult(U) instead would stall S(U+1) behind the exp->mult chain and
        # serialize the whole pipeline (~2us/unit instead of ~1.1us).
        # Head i-1's normalization is interleaved so engines never idle
        # between heads.
        norm_state = {}

        for i in range(hpc):
            qv, kv = head_aps(i)
            ot = ps_ot.tile([66, n], DT.float32, tag="ot")
            pending_ot = None

            def flush_ot():
                nonlocal pending_ot
                if pending_ot is None:
                    return
                e1, j, ho = pending_ot
                for (fo, fs) in hch:
                    nc.tensor.matmul(
                        ot[:, ho + fo:ho + fo + fs],
                        lhsT=v_s[:, j, (d + 2) * i:(d + 2) * i + d + 2],
                        rhs=e1[:, fo:fo + fs],
                        start=(j == 0), stop=(j == nt - 1),
                    )
                pending_ot = None

            for j in range(nt):
                eb_t = ebp.tile([128, n], DT.bfloat16, tag="eb")
                eb_dma = nc.sync.dma_start(
                    out=eb_t, in_=eb[i, 128 * j:128 * j + 128, :])
                if i == 0 and j < 4 and last_in_dma is not None:
                    # don't let eb prefetch contend with the x/w prologue loads
                    add_dep_helper(eb_dma.ins, last_in_dma.ins, sync=False,
                                   reason="dma priority")
                for h2 in range(n // hn):
                    ho = h2 * hn
                    st = ps_st.tile([128, hn], DT.float32, tag="st")
                    for (fo, fs) in hch:
                        nc.tensor.matmul(
                            st[:, fo:fo + fs],
                            lhsT=kv[:, 128 * j:128 * j + 128],
                            rhs=qv[:, ho + fo:ho + fo + fs],
                            start=True, stop=True,
                        )
                    e0 = e0p.tile([128, hn], DT.bfloat16, tag="e0")
                    nc.scalar.activation(e0, st, AF.Exp)
                    e1 = e1p.tile([128, hn], DT.bfloat16, tag="e1")
                    nc.vector.tensor_tensor(e1, e0, eb_t[:, ho:ho + hn],
                                            AluOpType.mult)
                    flush_ot()
                    pending_ot = (e1, j, ho)
                # interleave previous head's normalization broadcasts/mult
                if i - 1 in norm_state:
                    rsb_prev = norm_state[i - 1]
                    if j == 2:
                        emit_norm_bcast(i - 1, rsb_prev, 0)
                    elif j == 4:
                        emit_norm_bcast(i - 1, rsb_prev, 1)
                    elif j == 6:
                        emit_norm_mult(i - 1, rsb_prev)
                        del norm_state[i - 1]
            flush_ot()
            norm_state[i] = emit_norm_head(i, ot)

        # tail: normalization of the last head (nothing left to hide it under)
        i = hpc - 1
        rsb_last = norm_state.pop(i)
        for h2 in range(n // hn):
            emit_norm_bcast(i, rsb_last, h2)
        emit_norm_mult(i, rsb_last)

        ps_ot.release()
        ps_st.release()

        # ---- phase F: partial projection ytT = pw.T @ onrm ----
        ps_pj = tc.alloc_tile_pool(name="ps_pj", bufs=2, space="PSUM")
        ytp = tc.alloc_tile_pool(name="ytp", bufs=2)
        for m in range(mo):
            ps = ps_pj.tile([128, n], DT.float32, tag="pj")
            # head-outer so consecutive matmuls share lhsT and the LDWEIGHTS
            # hides under the previous matmul's stream
            for i in range(hpc):
                for (fo, fs) in qch:
                    nc.tensor.matmul(
                        ps[:, fo:fo + fs],
                        lhsT=pw_s[:, i, 128 * m:128 * m + 128],
                        rhs=osum[i][:, fo:fo + fs],
                        start=(i == 0), stop=(i == hpc - 1),
                    )
            yts = ytp.tile([128, n], DT.float32, tag="yts")
            nc.vector.tensor_copy(yts, ps)
            nc.sync.dma_start(out=yt[128 * m:128 * m + 128, :], in_=yts)

        ps_pj.release()
        ytp.release()
        normp.release()
        e1p.release()
        e0p.release()
        ebp.release()
        qkvout.release()
        persist.release()

    nc.compile()
    return nc


_PROG = {}


def _get_program(**kw):
    key = tuple(sorted(kw.items()))
    if key not in _PROG:
        _PROG[key] = build_program(**kw)
    return _PROG[key]


def make_in_maps(x, mask, qkv_w, qkv_b, rel_bias, proj_w):
    """Host-side shard + layout prep. Returns list of per-core input dicts."""
    x = np.asarray(x, dtype=np.float32)
    mask = np.asarray(mask)
    qkv_w = np.asarray(qkv_w, dtype=np.float32)
    qkv_b = np.asarray(qkv_b, dtype=np.float32)
    rel_bias = np.asarray(rel_bias, dtype=np.float32)
    proj_w = np.asarray(proj_w, dtype=np.float32)

    n_qk_chunks = 2 * ((HPC + 1) // 2)
    wqk_cols = 128 * n_qk_chunks
    wv_cols = HPC * (D + 2)
    has_bias = bool(np.any(qkv_b))
    c_in = C + 1 if has_bias else C

    # per-batch transposed activations
    xts = []
    for b in range(B):
        xb = x[b].T  # [C, N]
        if has_bias:
            xb = np.concatenate([xb, np.ones((1, N), np.float32)], axis=0)
        xts.append(np.ascontiguousarray(xb))

    maps = []
    for core in range(NCORES):
        b = core // 4
        heads = [HPC * (core % 4) + i for i in range(HPC)]

        wqk = np.zeros((c_in, wqk_cols), np.float32)
        wv = np.zeros((c_in, wv_cols), np.float32)
        pwm = np.zeros((HPC * D, C), np.float32)
        for i, h in enumerate(heads):
            base = 128 * (2 * (i // 2)) + 64 * (i % 2)
            wqk[:C, base:base + 64] = qkv_w[D * h:D * h + D, :].T * SCALE
            kbase = 128 * (2 * (i // 2) + 1) + 64 * (i % 2)
            wqk[:C, kbase:kbase + 64] = qkv_w[C + D * h:C + D * h + D, :].T
            wv[:C, (D + 2) * i:(D + 2) * i + D] = qkv_w[2 * C + D * h:2 * C + D * h + D, :].T
            if has_bias:
                wqk[C, base:base + 64] = qkv_b[D * h:D * h + D] * SCALE
                wqk[C, kbase:kbase + 64] = qkv_b[C + D * h:C + D * h + D]
                wv[C, (D + 2) * i:(D + 2) * i + D] = qkv_b[2 * C + D * h:2 * C + D * h + D]
            pwm[64 * i:64 * i + 64, :] = proj_w[:, D * h:D * h + D].T

        ebs = np.empty((HPC, N, N), ml_dtypes.bfloat16)
        mb = (mask[b, 0] != 0)
        for i, h in enumerate(heads):
            ebs[i] = (np.exp(rel_bias[h]) * mb).T.astype(ml_dtypes.bfloat16)

        maps.append({
            "xt": xts[b].astype(ml_dtypes.bfloat16),
            "wqk": wqk.astype(ml_dtypes.bfloat16),
            "wv": wv.astype(ml_dtypes.bfloat16),
            "eb": ebs,
            "pw": pwm.astype(ml_dtypes.bfloat16),
        })
    return maps, has_bias


def kernel(x, mask, qkv_w, qkv_b, rel_bias, proj_w, proj_b):
    global LAST_RESULTS
    maps, has_bias = make_in_maps(x, mask, qkv_w, qkv_b, rel_bias, proj_w)
    nc = _get_program(c_in=C + 1 if has_bias else C)

    trace = bool(os.environ.get("KERNEL_TRACE"))
    try:
        res = run_bass_kernel_spmd(
            nc, maps, list(range(NCORES)),
            trace=trace,
            trace_cores=list(range(NCORES)) if trace else None,
        )
    except Exception:
        if not trace:
            raise
        # tracing infra unavailable; rerun untraced
        os.environ["BASS_NEVER_TRACE"] = "1"
        res = run_bass_kernel_spmd(nc, maps, list(range(NCORES)), trace=False)
    LAST_RESULTS = res

    proj_b = np.asarray(proj_b, dtype=np.float32)
    out = np.empty((B, N, C), np.float32)
    for b in range(B):
        acc = res.results[4 * b]["yt"].astype(np.float32)
        for c in range(4 * b + 1, 4 * b + 4):
            acc = acc + res.results[c]["yt"]
        out[b] = acc.T + proj_b[None, :]
    return out


# revision 21
# speedup vs baseline: 1.3760x; 1.2185x over previous
"""Trainium2 Bass kernel for a 12-head MHA layer with relative position bias
and a 0/1 attention mask (B=2, N=2048, C=768, H=12, d=64), sharded over 8
NeuronCores (batch x head-group parallel: core c handles batch c//4 and heads
3*(c%4) .. 3*(c%4)+2).

Device math per core (heads i = 0..2, all in transposed "T" layouts):
  qT = (Wq*s).T^T @ xT         (s = d^-0.5 folded into Wq on host)
  ST[k,q]  = kT.T @ qT         (PSUM; k-tile PAIRS via PE row-tiling: tile jA
                                computes in array rows 0-63, jB in rows 64-127,
                                concurrently -- halves the S stream cycles)
  E[k,q]   = exp(ST) * EBT     (ACT exp to bf16 + DVE 2x-mode mult;
                                EBT = (exp(rel_bias)*mask).T in bf16)
  otT[d',q]= sum_k v'[k,d'] E[k,q]   (v' = [v | ones] -> row 64 = softmax denom)
  onrm     = otT[0:64] * (1/denom)   (recip on DVE via [16,128] spread, PE
                                      outer-product broadcast, interleaved into
                                      the next head's loop)
  ytT      = pwT.T @ [onrm0;onrm1] (+ pw2T.T @ onrm2)   (heads 0,1 packed into
                                      full-128-contraction matmuls; host sums
                                      partial yt across cores)

Scheduling: the exp stream starts ~17us in (B computes only heads 0/1's q,k
before attention begins; phase C / B-tail emit while ACT crunches, with the
OT matmuls deferred into a backlog). OT emission lags ~2 units behind so the
in-order PE queue always has dependency-ready work. The ACT engine runs only
exp (one table-set load, warmed at t=0).
"""

import os
from collections import deque

import numpy as np
import ml_dtypes

import concourse.bass as bass
import concourse.tile as tile
from concourse.tile import add_dep_helper
from concourse import bacc, mybir
from concourse.alu_op_type import AluOpType
from concourse.bass_utils import run_bass_kernel_spmd

AF = mybir.ActivationFunctionType
DT = mybir.dt
F32R = mybir.dt.float32r

B, N, C, H, D = 2, 2048, 768, 12, 64
HPC = H // 4          # heads per core (8 cores = 2 batches x 4 head-groups)
NCORES = 8
SCALE = float(D) ** -0.5

LAST_RESULTS = None   # BassKernelResults of the most recent kernel() call


def _q_chunks(n):
    """Split n into <=512 free-dim chunks for matmul moving operands."""
    out = []
    o = 0
    while o < n:
        sz = min(512, n - o)
        out.append((o, sz))
        o += sz
    return out


def build_program(n=N, c_in=C, hpc=HPC, d=D, c_out=C):
    """Build the per-core Bass/Tile program. Same program runs on all cores
    (SPMD); per-core data differs via in_maps."""
    nt = n // 128                       # number of 128-row k-tiles
    np_ = nt // 2                       # k-tile pairs
    qch = _q_chunks(n)
    ck = (c_in + 127) // 128            # contraction chunks over c_in
    # wqk columns: [q0|q1], [k0|k1], [q2|q2], [k2|k2] in 128-col m-chunks.
    # Head 2's q/k are host-duplicated into both partition halves; heads 0/1
    # get their opposite-half duplicates via SBUF DMA (qk_d) so every head has
    # q and k at partition bases 0 AND 64 (needed for S row-tiling pairs).
    n_qk_chunks = 2 * ((hpc + 1) // 2)  # 4 for hpc=3
    wqk_cols = 128 * n_qk_chunks
    wv_cols = hpc * (d + 2)             # [v_i | ones | pad] per head (even stride for fp32r)
    mo = c_out // 128                   # proj output row chunks

    def pc(kc):
        return min(128, c_in - 128 * kc)

    nc = bacc.Bacc("TRN2", target_bir_lowering=False, debug=False)
    xt = nc.dram_tensor("xt", [c_in, n], DT.bfloat16, kind="ExternalInput").ap()
    wqk = nc.dram_tensor("wqk", [c_in, wqk_cols], DT.bfloat16, kind="ExternalInput").ap()
    wv = nc.dram_tensor("wv", [c_in, wv_cols], DT.bfloat16, kind="ExternalInput").ap()
    eb = nc.dram_tensor("eb", [hpc, n, n], DT.bfloat16, kind="ExternalInput").ap()
    pw = nc.dram_tensor("pw", [hpc * d, c_out], DT.bfloat16, kind="ExternalInput").ap()
    yt = nc.dram_tensor("yt", [c_out, n], DT.float32, kind="ExternalOutput").ap()

    with tile.TileContext(nc) as tc:
        persist = tc.alloc_tile_pool(name="persist", bufs=1)
        qkvout = tc.alloc_tile_pool(name="qkvout", bufs=1)
        # attention pools are allocated up-front: early attention units run
        # while phase B/C matmuls are still being emitted. loadp is allocated
        # LAST so it can release first (pool releases are LIFO). PSUM: ps_st
        # gets banks 0-3, ps_qkv banks 4-5; ps_ot takes 4-7 after ps_qkv
        # releases.
        ebp = tc.alloc_tile_pool(name="ebp", bufs=9)
        e0p = tc.alloc_tile_pool(name="e0p", bufs=4)
        e1p = tc.alloc_tile_pool(name="e1p", bufs=16)
        normp = tc.alloc_tile_pool(name="normp", bufs=1)
        loadp = tc.alloc_tile_pool(name="loadp", bufs=1)
        ps_st = tc.alloc_tile_pool(name="ps_st", bufs=2, space="PSUM")
        ps_qkv = tc.alloc_tile_pool(name="ps_qkv", bufs=2, space="PSUM")

        pw01 = persist.tile([128, c_out], DT.bfloat16, tag="pw01")
        pw2 = persist.tile([64, c_out], DT.bfloat16, tag="pw2")
        ones_s = persist.tile([1, 128], DT.float32, tag="ones")
        nc.vector.memset(ones_s, 1.0)
        ones3 = persist.tile([128, hpc], DT.float32, tag="ones3")
        nc.vector.memset(ones3, 1.0)
        ones_r = persist.tile([1, 128], F32R, tag="ones_r")
        nc.vector.tensor_copy(ones_r, ones_s)
        # masked broadcast rows: [1,0] and [0,1] per 64-partition half (lets
        # two accumulating full-array matmuls fill rsb01's two halves without
        # col-offset tile_position, which walrus rejects)
        zer_s = persist.tile([1, 128], DT.float32, tag="zer_s")
        nc.vector.memset(zer_s, 0.0)
        ones_lo = persist.tile([1, 128], F32R, tag="ones_lo")
        nc.vector.tensor_copy(ones_lo[:, 0:64], ones_s[:, 0:64])
        nc.vector.tensor_copy(ones_lo[:, 64:128], zer_s[:, 64:128])
        ones_hi = persist.tile([1, 128], F32R, tag="ones_hi")
        nc.vector.tensor_copy(ones_hi[:, 0:64], zer_s[:, 0:64])
        nc.vector.tensor_copy(ones_hi[:, 64:128], ones_s[:, 64:128])
        # warm the ACT exp table-set at t=0 so the first real exp pays no
        # ACT_TABLE_LOAD (~2.7us) on the critical path
        wtmp = persist.tile([1, 128], DT.float32, tag="wtmp")
        nc.scalar.activation(wtmp, ones_s, AF.Exp)
        nc.gpsimd.dma_start(out=pw01, in_=pw[0:128, :])
        nc.gpsimd.dma_start(out=pw2, in_=pw[128:128 + 64, :])

        qk_s = qkvout.tile([128, n_qk_chunks, n], DT.bfloat16, tag="qk")
        qk_d = qkvout.tile([128, 2, n], DT.bfloat16, tag="qkd")
        v_s = qkvout.tile([128, nt, wv_cols], DT.bfloat16, tag="v")

        xts = loadp.tile([128, ck, n], DT.bfloat16, tag="xts")
        wqk_s = loadp.tile([128, ck, wqk_cols], DT.bfloat16, tag="wqk")
        wv_s = loadp.tile([128, ck, wv_cols], DT.bfloat16, tag="wv")
        last_in_dma = None
        for kc in range(ck):
            p = pc(kc)
            nc.gpsimd.dma_start(out=xts[:p, kc, :], in_=xt[128 * kc:128 * kc + p, :])
            nc.gpsimd.dma_start(out=wqk_s[:p, kc, :], in_=wqk[128 * kc:128 * kc + p, :])
            last_in_dma = nc.gpsimd.dma_start(
                out=wv_s[:p, kc, :], in_=wv[128 * kc:128 * kc + p, :])

        # ---- phase B: qkT = wqk.T @ xT  -> qk_s ----
        def emit_qk_chunk(m):
            for (fo, fs) in qch:
                ps = ps_qkv.tile([128, 512], DT.float32, tag="psqkv", name=f"psB{m}")
                for kc in range(ck):
                    p = pc(kc)
                    nc.tensor.matmul(
                        ps[:, :fs],
                        lhsT=wqk_s[:p, kc, 128 * m:128 * m + 128],
                        rhs=xts[:p, kc, fo:fo + fs],
                        start=(kc == 0), stop=(kc == ck - 1),
                    )
                nc.vector.tensor_copy(qk_s[:, m, fo:fo + fs], ps[:, :fs])

        for m in range(min(2, n_qk_chunks)):
            emit_qk_chunk(m)

        # heads 0/1 opposite-half duplicates: qk_d[:,0] = [q1|q0],
        # qk_d[:,1] = [k1|k0] (SBUF->SBUF partition swap of m0/m1)
        for m in range(2):
            nc.sync.dma_start(out=qk_d[64:128, m, :], in_=qk_s[0:64, m, :])
            nc.sync.dma_start(out=qk_d[0:64, m, :], in_=qk_s[64:128, m, :])

        def emit_c_chunk(j):
            ps = ps_qkv.tile([128, wv_cols], DT.float32, tag="psqkv")
            for kc in range(ck):
                p = pc(kc)
                nc.tensor.matmul(
                    ps,
                    lhsT=xts[:p, kc, 128 * j:128 * j + 128],
                    rhs=wv_s[:p, kc, :],
                    start=(kc == 0), stop=(kc == ck - 1),
                )
            nc.vector.tensor_copy(v_s[:, j, :], ps)
            nc.vector.tensor_copy(
                v_s[:, j, :].rearrange("p (h c) -> p h c", c=d + 2)[:, :, d],
                ones3)

        # ---- per-head q/k access at both partition bases ----
        def head_aps(i):
            if i < 2:
                mq, mk = 0, 1
                if i == 0:
                    q0 = qk_s[0:64, mq, :]
                    q64 = qk_d[64:128, mq, :]
                    k0 = qk_s[0:64, mk, :]
                    k64 = qk_d[64:128, mk, :]
                else:
                    q0 = qk_d[0:64, mq, :]
                    q64 = qk_s[64:128, mq, :]
                    k0 = qk_d[0:64, mk, :]
                    k64 = qk_s[64:128, mk, :]
            else:
                q0 = qk_s[0:64, 2, :]
                q64 = qk_s[64:128, 2, :]
                k0 = qk_s[0:64, 3, :]
                k64 = qk_s[64:128, 3, :]
            return q0, q64, k0, k64

        hn = n // 2
        hch = _q_chunks(hn)

        osum01 = normp.tile([128, n], DT.bfloat16, tag="osum01")
        osum2 = normp.tile([64, n], DT.bfloat16, tag="osum2")
        tmp01 = normp.tile([64, n], DT.bfloat16, tag="tmp01")
        # dn is consumed (spread-DMA'd) right after each head; rrow0/rrow1
        # stay live until the paired broadcast during head 2
        dn_t = normp.tile([1, n], DT.float32, tag="dn", name="dn")
        rrow_ts = [normp.tile([1, n], F32R, tag=f"rrow{i}", name=f"rrow{i}")
                   for i in range(hpc)]
        rsb01 = normp.tile([128, n], DT.float32, tag="rsb01")
        rsb2 = normp.tile([64, n], DT.float32, tag="rsb2")

        # --- softmax normalization (ACT-free: DVE recip + PE bcast) ---
        def emit_norm_head(i, ot):
            nc.vector.tensor_copy(dn_t, ot[64:65, :])
            if i == 0:
                nc.vector.tensor_copy(osum01[0:64, :], ot[0:64, :])
            elif i == 1:
                # partition-shift into osum01's upper half goes through DMA
                nc.vector.tensor_copy(tmp01, ot[0:64, :])
                nc.sync.dma_start(out=osum01[64:128, :], in_=tmp01)
            else:
                nc.vector.tensor_copy(osum2, ot[0:64, :])
            dsp = normp.tile([16, n // 16], DT.float32, tag="dsp", name=f"dsp{i}")
            rsp = normp.tile([16, n // 16], DT.float32, tag="rsp", name=f"rsp{i}")
            rspr = normp.tile([16, n // 16], F32R, tag="rspr", name=f"rspr{i}")
            nc.sync.dma_start(out=dsp, in_=dn_t)
            nc.vector.reciprocal(rsp, dsp)
            nc.vector.tensor_copy(rspr, rsp)
            nc.sync.dma_start(out=rrow_ts[i], in_=rspr)

        def emit_norm_bcast01(h2):
            """Packed broadcast: rows 0-63 get 1/den0, rows 64-127 get
            1/den1, via two accumulating full-array outer products."""
            ho = h2 * hn
            rps = ps_st.tile([128, hn], DT.float32, tag="st", name=f"rps01_{h2}")
            for (fo, fs) in hch:
                nc.tensor.matmul(
                    rps[:, fo:fo + fs],
                    lhsT=ones_lo,
                    rhs=rrow_ts[0][:, ho + fo:ho + fo + fs],
                    start=True, stop=False,
                )
                nc.tensor.matmul(
                    rps[:, fo:fo + fs],
                    lhsT=ones_hi,
                    rhs=rrow_ts[1][:, ho + fo:ho + fo + fs],
                    start=False, stop=True,
                )
            nc.vector.tensor_copy(rsb01[:, ho:ho + hn], rps)

        def emit_norm_bcast2(h2):
            ho = h2 * hn
            rps = ps_st.tile([128, hn], DT.float32, tag="st", name=f"rps2_{h2}")
            for (fo, fs) in hch:
                nc.tensor.matmul(
                    rps[0:64, fo:fo + fs],
                    lhsT=ones_r[0:1, 0:64],
                    rhs=rrow_ts[2][:, ho + fo:ho + fo + fs],
                    start=True, stop=True,
                )
            nc.vector.tensor_copy(rsb2[:, ho:ho + hn], rps[0:64, :])

        def emit_norm_mult01():
            # one 128-partition multiply covers heads 0 and 1
            nc.vector.tensor_tensor(osum01, osum01, rsb01, AluOpType.mult)

        def emit_norm_mult2():
            nc.vector.tensor_tensor(osum2, osum2, rsb2, AluOpType.mult)

        # ---- phase D: per-head attention ----
        # Each k-tile j is ONE row-tiled PE pair: array rows 0-63 compute
        # S for q-columns 0..hn-1 (stA) while rows 64-127 compute q-columns
        # hn..n-1 (stB), concurrently -- the full [128, n] score tile streams
        # in n/2 column-cycles. The exp->mult->OT tail is deferred 2 k-tiles
        # (the pending deque) and OT matmuls are flushed BEFORE the next S so
        # the in-order PE queue always has dependency-ready work ahead of the
        # st-slot waits.
        pending = deque()

        def flush_pending(keep=0):
            while len(pending) > keep:
                e1A, e1B, i_, j_, ot_ = pending.popleft()
                for half, e1 in ((0, e1A), (1, e1B)):
                    for (fo, fs) in hch:
                        nc.tensor.matmul(
                            ot_[:, half * hn + fo:half * hn + fo + fs],
                            lhsT=v_s[:, j_, (d + 2) * i_:(d + 2) * i_ + d + 2],
                            rhs=e1[:, fo:fo + fs],
                            start=(j_ == 0), stop=(j_ == nt - 1),
                        )

        def emit_j(i, ot, j, q0, q64, k0, k64, keep):
            """OT flush (lagged), then S (row-tiled h2 pair) + exp + mult
            for k-tile j."""
            flush_pending(keep)
            eb_t = ebp.tile([128, n], DT.bfloat16, tag="eb")
            eb_dma = nc.sync.dma_start(out=eb_t, in_=eb[i, 128 * j:128 * j + 128, :])
            if i == 0 and j < 8 and last_in_dma is not None:
                add_dep_helper(eb_dma.ins, last_in_dma.ins, sync=False,
                               reason="dma priority")
            stA = ps_st.tile([128, hn], DT.float32, tag="st", name="stA")
            stB = ps_st.tile([128, hn], DT.float32, tag="st", name="stB")
            for (fo, fs) in hch:
                nc.tensor.matmul(
                    stA[:, fo:fo + fs],
                    lhsT=k0[:, 128 * j:128 * j + 128],
                    rhs=q0[:, fo:fo + fs],
                    start=True, stop=True,
                )
                nc.tensor.matmul(
                    stB[:, fo:fo + fs],
                    lhsT=k64[:, 128 * j:128 * j + 128],
                    rhs=q64[:, hn + fo:hn + fo + fs],
                    start=True, stop=True,
                )
            e0A = e0p.tile([128, hn], DT.bfloat16, tag="e0")
            nc.scalar.activation(e0A, stA, AF.Exp)
            e1A = e1p.tile([128, hn], DT.bfloat16, tag="e1")
            nc.vector.tensor_tensor(e1A, e0A, eb_t[:, 0:hn], AluOpType.mult)
            e0B = e0p.tile([128, hn], DT.bfloat16, tag="e0")
            nc.scalar.activation(e0B, stB, AF.Exp)
            e1B = e1p.tile([128, hn], DT.bfloat16, tag="e1")
            nc.vector.tensor_tensor(e1B, e0B, eb_t[:, hn:n], AluOpType.mult)
            pending.append((e1A, e1B, i, j, ot))

        ps_ot_holder = {}

        def get_ot():
            if "pool" not in ps_ot_holder:
                ps_ot_holder["pool"] = tc.alloc_tile_pool(
                    name="ps_ot", bufs=1, space="PSUM")
            return ps_ot_holder["pool"].tile([66, n], DT.float32, tag="ot",
                                             name="ot")

        # -- head 0, early section: S/exp/mult only (OT backlog builds while
        # phase C and the B tail are emitted; the ot PSUM tile can only exist
        # after ps_qkv releases banks 4-5, so the backlog carries ot=None) --
        q0, q64, k0, k64 = head_aps(0)
        early = min(7, max(nt - 2, 1))
        c_done = 0
        for j in range(early):
            emit_j(0, None, j, q0, q64, k0, k64, keep=len(pending) + 1)
            # phase C rides along so the PE stays fed between S pairs
            for _ in range(2):
                if c_done < nt:
                    emit_c_chunk(c_done)
                    c_done += 1
        while c_done < nt:
            emit_c_chunk(c_done)
            c_done += 1
        for m in range(min(2, n_qk_chunks), n_qk_chunks):
            emit_qk_chunk(m)

        loadp.release()
        ps_qkv.release()

        ot = get_ot()
        # patch the backlog's ot (deferred entries carry ot=None)
        pending_fixed = deque((a, b, i_, j_, ot) for (a, b, i_, j_, _)
                              in pending)
        pending.clear()
        pending.extend(pending_fixed)

        for i in range(hpc):
            q0, q64, k0, k64 = head_aps(i)
            if i > 0:
                ot = get_ot()
            start_j = early if i == 0 else 0
            for j in range(start_j, nt):
                emit_j(i, ot, j, q0, q64, k0, k64, keep=2)
                # heads 0+1's packed normalization rides inside head 2's loop
                # (it needs both rrow0 and rrow1)
                if i == 2:
                    if j == 2:
                        emit_norm_bcast01(0)
                    elif j == 4:
                        emit_norm_bcast01(1)
                    elif j == 6:
                        emit_norm_mult01()
            flush_pending(0)
            emit_norm_head(i, ot)

        # tail: head 2's normalization
        for h2 in range(n // hn):
            emit_norm_bcast2(h2)
        emit_norm_mult2()

        ps_ot_holder["pool"].release()
        ps_st.release()

        # ---- phase F: ytT = pw01.T @ osum01 + pw2.T @ osum2 ----
        ps_pj = tc.alloc_tile_pool(name="ps_pj", bufs=2, space="PSUM")
        ytp = tc.alloc_tile_pool(name="ytp", bufs=2)
        for m in range(mo):
            ps = ps_pj.tile([128, n], DT.float32, tag="pj")
            for (fo, fs) in qch:
                nc.tensor.matmul(
                    ps[:, fo:fo + fs],
                    lhsT=pw01[:, 128 * m:128 * m + 128],
                    rhs=osum01[:, fo:fo + fs],
                    start=True, stop=False,
                )
            for (fo, fs) in qch:
                nc.tensor.matmul(
                    ps[:, fo:fo + fs],
                    lhsT=pw2[:, 128 * m:128 * m + 128],
                    rhs=osum2[:, fo:fo + fs],
                    start=False, stop=True,
                )
            yts = ytp.tile([128, n], DT.float32, tag="yts")
            nc.vector.tensor_copy(yts, ps)
            nc.sync.dma_start(out=yt[128 * m:128 * m + 128, :], in_=yts)

        ps_pj.release()
        ytp.release()
        normp.release()
        e1p.release()
        e0p.release()
        ebp.release()
        qkvout.release()
        persist.release()

    nc.compile()
    return nc


_PROG = {}


def _get_program(**kw):
    key = tuple(sorted(kw.items()))
    if key not in _PROG:
        _PROG[key] = build_program(**kw)
    return _PROG[key]


def make_in_maps(x, mask, qkv_w, qkv_b, rel_bias, proj_w):
    """Host-side shard + layout prep. Returns list of per-core input dicts."""
    x = np.asarray(x, dtype=np.float32)
    mask = np.asarray(mask)
    qkv_w = np.asarray(qkv_w, dtype=np.float32)
    qkv_b = np.asarray(qkv_b, dtype=np.float32)
    rel_bias = np.asarray(rel_bias, dtype=np.float32)
    proj_w = np.asarray(proj_w, dtype=np.float32)

    n_qk_chunks = 2 * ((HPC + 1) // 2)
    wqk_cols = 128 * n_qk_chunks
    wv_cols = HPC * (D + 2)
    has_bias = bool(np.any(qkv_b))
    c_in = C + 1 if has_bias else C

    xts = []
    for b in range(B):
        xb = x[b].T  # [C, N]
        if has_bias:
            xb = np.concatenate([xb, np.ones((1, N), np.float32)], axis=0)
        xts.append(np.ascontiguousarray(xb))

    maps = []
    for core in range(NCORES):
        b = core // 4
        heads = [HPC * (core % 4) + i for i in range(HPC)]

        wqk = np.zeros((c_in, wqk_cols), np.float32)
        wv = np.zeros((c_in, wv_cols), np.float32)
        pwm = np.zeros((HPC * D, C), np.float32)
        for i, h in enumerate(heads):
            qw = qkv_w[D * h:D * h + D, :].T * SCALE
            kw = qkv_w[C + D * h:C + D * h + D, :].T
            if i < 2:
                base = 64 * i          # m0 = [q0|q1]
                kbase = 128 + 64 * i   # m1 = [k0|k1]
                wqk[:C, base:base + 64] = qw
                wqk[:C, kbase:kbase + 64] = kw
            else:
                # m2 = [q2|q2], m3 = [k2|k2] (duplicated for S row-tiling)
                wqk[:C, 256:256 + 64] = qw
                wqk[:C, 320:320 + 64] = qw
                wqk[:C, 384:384 + 64] = kw
                wqk[:C, 448:448 + 64] = kw
            wv[:C, (D + 2) * i:(D + 2) * i + D] = qkv_w[2 * C + D * h:2 * C + D * h + D, :].T
            if has_bias:
                qb = qkv_b[D * h:D * h + D] * SCALE
                kb = qkv_b[C + D * h:C + D * h + D]
                if i < 2:
                    wqk[C, 64 * i:64 * i + 64] = qb
                    wqk[C, 128 + 64 * i:128 + 64 * i + 64] = kb
                else:
                    wqk[C, 256:256 + 64] = qb
                    wqk[C, 320:320 + 64] = qb
                    wqk[C, 384:384 + 64] = kb
                    wqk[C, 448:448 + 64] = kb
                wv[C, (D + 2) * i:(D + 2) * i + D] = qkv_b[2 * C + D * h:2 * C + D * h + D]
            pwm[64 * i:64 * i + 64, :] = proj_w[:, D * h:D * h + D].T

        ebs = np.empty((HPC, N, N), ml_dtypes.bfloat16)
        mb = (mask[b, 0] != 0)
        for i, h in enumerate(heads):
            ebs[i] = (np.exp(rel_bias[h]) * mb).T.astype(ml_dtypes.bfloat16)

        maps.append({
            "xt": xts[b].astype(ml_dtypes.bfloat16),
            "wqk": wqk.astype(ml_dtypes.bfloat16),
            "wv": wv.astype(ml_dtypes.bfloat16),
            "eb": ebs,
            "pw": pwm.astype(ml_dtypes.bfloat16),
        })
    return maps, has_bias


def kernel(x, mask, qkv_w, qkv_b, rel_bias, proj_w, proj_b):
    global LAST_RESULTS
    maps, has_bias = make_in_maps(x, mask, qkv_w, qkv_b, rel_bias, proj_w)
    nc = _get_program(c_in=C + 1 if has_bias else C)

    trace = bool(os.environ.get("KERNEL_TRACE"))
    try:
        res = run_bass_kernel_spmd(
            nc, maps, list(range(NCORES)),
            trace=trace,
            trace_cores=list(range(NCORES)) if trace else None,
        )
    except Exception:
        if not trace:
            raise
        os.environ["BASS_NEVER_TRACE"] = "1"
        res = run_bass_kernel_spmd(nc, maps, list(range(NCORES)), trace=False)
    LAST_RESULTS = res

    proj_b = np.asarray(proj_b, dtype=np.float32)
    out = np.empty((B, N, C), np.float32)
    for b in range(B):
        acc = res.results[4 * b]["yt"].astype(np.float32)
        for c in range(4 * b + 1, 4 * b + 4):
            acc = acc + res.results[c]["yt"]
        out[b] = acc.T + proj_b[None, :]
    return out


# revision 24
# speedup vs baseline: 1.5572x; 1.1317x over previous
"""Trainium2 Bass kernel for a 12-head MHA layer with relative position bias
and a 0/1 attention mask (B=2, N=2048, C=768, H=12, d=64), sharded over 8
NeuronCores (batch x head-group parallel: core c handles batch c//4 and heads
3*(c%4) .. 3*(c%4)+2).

Device math per core (heads i = 0..2, all in transposed "T" layouts):
  qT = (Wq*s).T^T @ xT         (s = d^-0.5 folded into Wq on host)
  ST[k,q]  = kT.T @ qT         (PSUM; k-tile PAIRS via PE row-tiling: tile jA
                                computes in array rows 0-63, jB in rows 64-127,
                                concurrently -- halves the S stream cycles)
  E[k,q]   = exp(ST) * EBT     (ACT exp to bf16 + DVE 2x-mode mult;
                                EBT = (exp(rel_bias)*mask).T in bf16)
  otT[d',q]= sum_k v'[k,d'] E[k,q]   (v' = [v | ones] -> row 64 = softmax denom)
  onrm     = otT[0:64] * (1/denom)   (recip on DVE via [16,128] spread, PE
                                      outer-product broadcast, interleaved into
                                      the next head's loop)
  ytT      = pwT.T @ [onrm0;onrm1] (+ pw2T.T @ onrm2)   (heads 0,1 packed into
                                      full-128-contraction matmuls; host sums
                                      partial yt across cores)

Scheduling: the exp stream starts ~17us in (B computes only heads 0/1's q,k
before attention begins; phase C / B-tail emit while ACT crunches, with the
OT matmuls deferred into a backlog). OT emission lags ~2 units behind so the
in-order PE queue always has dependency-ready work. The ACT engine runs only
exp (one table-set load, warmed at t=0).
"""

import os
from collections import deque

import numpy as np
import ml_dtypes

import concourse.bass as bass
import concourse.tile as tile
from concourse.tile import add_dep_helper
from concourse import bacc, mybir
from concourse.alu_op_type import AluOpType
from concourse.bass_utils import run_bass_kernel_spmd

AF = mybir.ActivationFunctionType
DT = mybir.dt
F32R = mybir.dt.float32r

B, N, C, H, D = 2, 2048, 768, 12, 64
HPC = H // 4          # heads per core (8 cores = 2 batches x 4 head-groups)
NCORES = 8
SCALE = float(D) ** -0.5

LAST_RESULTS = None   # BassKernelResults of the most recent kernel() call


def _q_chunks(n):
    """Split n into <=512 free-dim chunks for matmul moving operands."""
    out = []
    o = 0
    while o < n:
        sz = min(512, n - o)
        out.append((o, sz))
        o += sz
    return out


def build_program(n=N, c_in=C, hpc=HPC, d=D, c_out=C):
    """Build the per-core Bass/Tile program. Same program runs on all cores
    (SPMD); per-core data differs via in_maps."""
    nt = n // 128                       # number of 128-row k-tiles
    np_ = nt // 2                       # k-tile pairs
    qch = _q_chunks(n)
    ck = (c_in + 127) // 128            # contraction chunks over c_in
    # wqk columns: [q0|q1], [k0|k1], [q2|q2], [k2|k2] in 128-col m-chunks.
    # Head 2's q/k are host-duplicated into both partition halves; heads 0/1
    # get their opposite-half duplicates via SBUF DMA (qk_d) so every head has
    # q and k at partition bases 0 AND 64 (needed for S row-tiling pairs).
    n_qk_chunks = 3                     # [q0|q1], [k0|k1], [q2|k2]
    wqk_cols = 128 * n_qk_chunks
    wv_cols = hpc * (d + 2)             # [v_i | ones | pad] per head (even stride for fp32r)
    mo = c_out // 128                   # proj output row chunks

    def pc(kc):
        return min(128, c_in - 128 * kc)

    nc = bacc.Bacc("TRN2", target_bir_lowering=False, debug=False)
    xt = nc.dram_tensor("xt", [c_in, n], DT.bfloat16, kind="ExternalInput").ap()
    wqk = nc.dram_tensor("wqk", [c_in, wqk_cols], DT.bfloat16, kind="ExternalInput").ap()
    wv = nc.dram_tensor("wv", [c_in, wv_cols], DT.bfloat16, kind="ExternalInput").ap()
    eb = nc.dram_tensor("eb", [hpc, n, n], DT.bfloat16, kind="ExternalInput").ap()
    pw = nc.dram_tensor("pw", [hpc * d, c_out], DT.bfloat16, kind="ExternalInput").ap()
    yt = nc.dram_tensor("yt", [c_out, n], DT.float32, kind="ExternalOutput").ap()

    with tile.TileContext(nc) as tc:
        persist = tc.alloc_tile_pool(name="persist", bufs=1)
        qkvout = tc.alloc_tile_pool(name="qkvout", bufs=1)
        # attention pools are allocated up-front: early attention units run
        # while phase B/C matmuls are still being emitted. loadp is allocated
        # LAST so it can release first (pool releases are LIFO). PSUM: ps_st
        # gets banks 0-3, ps_qkv banks 4-5; ps_ot takes 4-7 after ps_qkv
        # releases.
        ebp = tc.alloc_tile_pool(name="ebp", bufs=9)
        e0p = tc.alloc_tile_pool(name="e0p", bufs=4)
        e1p = tc.alloc_tile_pool(name="e1p", bufs=16)
        normp = tc.alloc_tile_pool(name="normp", bufs=1)
        loadp = tc.alloc_tile_pool(name="loadp", bufs=1)
        ps_st = tc.alloc_tile_pool(name="ps_st", bufs=2, space="PSUM")
        ps_qkv = tc.alloc_tile_pool(name="ps_qkv", bufs=2, space="PSUM")

        pw01 = persist.tile([128, c_out], DT.bfloat16, tag="pw01")
        pw2 = persist.tile([64, c_out], DT.bfloat16, tag="pw2")
        ones_s = persist.tile([1, 128], DT.float32, tag="ones")
        nc.vector.memset(ones_s, 1.0)
        ones3 = persist.tile([128, hpc], DT.float32, tag="ones3")
        nc.vector.memset(ones3, 1.0)
        ones_r = persist.tile([1, 128], F32R, tag="ones_r")
        nc.vector.tensor_copy(ones_r, ones_s)
        # masked broadcast rows: [1,0] and [0,1] per 64-partition half (lets
        # two accumulating full-array matmuls fill rsb01's two halves without
        # col-offset tile_position, which walrus rejects)
        zer_s = persist.tile([1, 128], DT.float32, tag="zer_s")
        nc.vector.memset(zer_s, 0.0)
        ones_lo = persist.tile([1, 128], F32R, tag="ones_lo")
        nc.vector.tensor_copy(ones_lo[:, 0:64], ones_s[:, 0:64])
        nc.vector.tensor_copy(ones_lo[:, 64:128], zer_s[:, 64:128])
        ones_hi = persist.tile([1, 128], F32R, tag="ones_hi")
        nc.vector.tensor_copy(ones_hi[:, 0:64], zer_s[:, 0:64])
        nc.vector.tensor_copy(ones_hi[:, 64:128], ones_s[:, 64:128])
        # warm the ACT exp table-set at t=0 so the first real exp pays no
        # ACT_TABLE_LOAD (~2.7us) on the critical path
        wtmp = persist.tile([1, 128], DT.float32, tag="wtmp")
        nc.scalar.activation(wtmp, ones_s, AF.Exp)
        nc.gpsimd.dma_start(out=pw01, in_=pw[0:128, :])
        nc.gpsimd.dma_start(out=pw2, in_=pw[128:128 + 64, :])

        qk_s = qkvout.tile([128, n_qk_chunks, n], DT.bfloat16, tag="qk")
        qk_d = qkvout.tile([128, 3, n], DT.bfloat16, tag="qkd")
        v_s = qkvout.tile([128, nt, wv_cols], DT.bfloat16, tag="v")

        xts = loadp.tile([128, ck, n], DT.bfloat16, tag="xts")
        wqk_s = loadp.tile([128, ck, wqk_cols], DT.bfloat16, tag="wqk")
        wv_s = loadp.tile([128, ck, wv_cols], DT.bfloat16, tag="wv")
        last_in_dma = None
        for kc in range(ck):
            p = pc(kc)
            nc.gpsimd.dma_start(out=xts[:p, kc, :], in_=xt[128 * kc:128 * kc + p, :])
            nc.gpsimd.dma_start(out=wqk_s[:p, kc, :], in_=wqk[128 * kc:128 * kc + p, :])
            last_in_dma = nc.gpsimd.dma_start(
                out=wv_s[:p, kc, :], in_=wv[128 * kc:128 * kc + p, :])

        # ---- phase B: qkT = wqk.T @ xT  -> qk_s ----
        def emit_qk_chunk(m):
            for (fo, fs) in qch:
                ps = ps_qkv.tile([128, 512], DT.float32, tag="psqkv", name=f"psB{m}")
                for kc in range(ck):
                    p = pc(kc)
                    nc.tensor.matmul(
                        ps[:, :fs],
                        lhsT=wqk_s[:p, kc, 128 * m:128 * m + 128],
                        rhs=xts[:p, kc, fo:fo + fs],
                        start=(kc == 0), stop=(kc == ck - 1),
                    )
                nc.vector.tensor_copy(qk_s[:, m, fo:fo + fs], ps[:, :fs])

        def emit_qk_fo(m, ci):
            fo, fs = qch[ci]
            ps = ps_qkv.tile([128, 512], DT.float32, tag="psqkv",
                             name=f"psB{m}_{ci}")
            for kc in range(ck):
                p = pc(kc)
                nc.tensor.matmul(
                    ps[:, :fs],
                    lhsT=wqk_s[:p, kc, 128 * m:128 * m + 128],
                    rhs=xts[:p, kc, fo:fo + fs],
                    start=(kc == 0), stop=(kc == ck - 1),
                )
            nc.vector.tensor_copy(qk_s[:, m, fo:fo + fs], ps[:, :fs])

        # head 0's tile-A S matmuls need m0 cols 0..hn-1 + m1's first k-tiles;
        # tile-B needs the rest of m0 plus its partition-swapped duplicate.
        # Emit in that order so the exp stream starts as early as possible.
        nq = len(qch)
        ha = max(1, (n // 2 + 511) // 512)       # m0 chunks covering cols<hn
        early_n = min(7, max(nt - 2, 1))
        kneed = min(nq, max(1, (early_n * 128 + 511) // 512))
        pre = ([(0, ci) for ci in range(ha)] + [(1, 0)] +
               [(0, ci) for ci in range(ha, nq)] +
               [(1, ci) for ci in range(1, kneed)])
        for m, ci in pre:
            emit_qk_fo(m, ci)
        # duplicates needed by head 0 (upper-half copies of q0 / k0)
        nc.sync.dma_start(out=qk_d[64:128, 0, :], in_=qk_s[0:64, 0, :])
        ecov = min(n, ((min(7, max(nt - 2, 1)) * 128 + 511) // 512) * 512)
        nc.sync.dma_start(out=qk_d[64:128, 1, 0:ecov],
                          in_=qk_s[0:64, 1, 0:ecov])

        def emit_c_chunk(j):
            ps = ps_qkv.tile([128, wv_cols], DT.float32, tag="psqkv")
            for kc in range(ck):
                p = pc(kc)
                nc.tensor.matmul(
                    ps,
                    lhsT=xts[:p, kc, 128 * j:128 * j + 128],
                    rhs=wv_s[:p, kc, :],
                    start=(kc == 0), stop=(kc == ck - 1),
                )
            nc.vector.tensor_copy(v_s[:, j, :], ps)
            nc.vector.tensor_copy(
                v_s[:, j, :].rearrange("p (h c) -> p h c", c=d + 2)[:, :, d],
                ones3)

        # ---- per-head q/k access at both partition bases ----
        def head_aps(i):
            if i < 2:
                mq, mk = 0, 1
                if i == 0:
                    q0 = qk_s[0:64, mq, :]
                    q64 = qk_d[64:128, mq, :]
                    k0 = qk_s[0:64, mk, :]
                    k64 = qk_d[64:128, mk, :]
                else:
                    q0 = qk_d[0:64, mq, :]
                    q64 = qk_s[64:128, mq, :]
                    k0 = qk_d[0:64, mk, :]
                    k64 = qk_s[64:128, mk, :]
            else:
                # m2 = [q2|k2]; qk_d slot 2 = [k2|q2]
                q0 = qk_s[0:64, 2, :]
                q64 = qk_d[64:128, 2, :]
                k0 = qk_d[0:64, 2, :]
                k64 = qk_s[64:128, 2, :]
            return q0, q64, k0, k64

        hn = n // 2
        hch = _q_chunks(hn)

        osum01 = normp.tile([128, n], DT.bfloat16, tag="osum01")
        osum2 = normp.tile([64, n], DT.bfloat16, tag="osum2")
        tmp01 = normp.tile([64, n], DT.bfloat16, tag="tmp01")
        # dn is consumed (spread-DMA'd) right after each head; rrow0/rrow1
        # stay live until the paired broadcast during head 2
        dn_t = normp.tile([1, n], DT.float32, tag="dn", name="dn")
        rrow_ts = [normp.tile([1, n], F32R, tag=f"rrow{i}", name=f"rrow{i}")
                   for i in range(hpc)]
        rsb01 = normp.tile([128, n], DT.float32, tag="rsb01")
        rsb2 = normp.tile([64, n], DT.float32, tag="rsb2")

        # --- softmax normalization (ACT-free: DVE recip + PE bcast) ---
        def emit_norm_head(i, ot):
            nc.vector.tensor_copy(dn_t, ot[64:65, :])
            if i == 0:
                nc.vector.tensor_copy(osum01[0:64, :], ot[0:64, :])
            elif i == 1:
                # partition-shift into osum01's upper half goes through DMA
                nc.vector.tensor_copy(tmp01, ot[0:64, :])
                nc.sync.dma_start(out=osum01[64:128, :], in_=tmp01)
            else:
                nc.vector.tensor_copy(osum2, ot[0:64, :])
            dsp = normp.tile([16, n // 16], DT.float32, tag="dsp", name=f"dsp{i}")
            rsp = normp.tile([16, n // 16], DT.float32, tag="rsp", name=f"rsp{i}")
            rspr = normp.tile([16, n // 16], F32R, tag="rspr", name=f"rspr{i}")
            nc.sync.dma_start(out=dsp, in_=dn_t)
            nc.vector.reciprocal(rsp, dsp)
            nc.vector.tensor_copy(rspr, rsp)
            nc.sync.dma_start(out=rrow_ts[i], in_=rspr)

        def emit_norm_bcast01(h2):
            """Packed broadcast: rows 0-63 get 1/den0, rows 64-127 get
            1/den1, via two accumulating full-array outer products."""
            ho = h2 * hn
            rps = ps_st.tile([128, hn], DT.float32, tag="st", name=f"rps01_{h2}")
            for (fo, fs) in hch:
                nc.tensor.matmul(
                    rps[:, fo:fo + fs],
                    lhsT=ones_lo,
                    rhs=rrow_ts[0][:, ho + fo:ho + fo + fs],
                    start=True, stop=False,
                )
                nc.tensor.matmul(
                    rps[:, fo:fo + fs],
                    lhsT=ones_hi,
                    rhs=rrow_ts[1][:, ho + fo:ho + fo + fs],
                    start=False, stop=True,
                )
            nc.vector.tensor_copy(rsb01[:, ho:ho + hn], rps)

        def emit_norm_bcast2(h2):
            ho = h2 * hn
            rps = ps_st.tile([128, hn], DT.float32, tag="st", name=f"rps2_{h2}")
            for (fo, fs) in hch:
                nc.tensor.matmul(
                    rps[0:64, fo:fo + fs],
                    lhsT=ones_r[0:1, 0:64],
                    rhs=rrow_ts[2][:, ho + fo:ho + fo + fs],
                    start=True, stop=True,
                )
            nc.vector.tensor_copy(rsb2[:, ho:ho + hn], rps[0:64, :])

        def emit_norm_mult01():
            # one 128-partition multiply covers heads 0 and 1
            nc.vector.tensor_tensor(osum01, osum01, rsb01, AluOpType.mult)

        def emit_norm_mult2():
            nc.vector.tensor_tensor(osum2, osum2, rsb2, AluOpType.mult)

        # ---- phase D: per-head attention ----
        # Each k-tile j is ONE row-tiled PE pair: array rows 0-63 compute
        # S for q-columns 0..hn-1 (stA) while rows 64-127 compute q-columns
        # hn..n-1 (stB), concurrently -- the full [128, n] score tile streams
        # in n/2 column-cycles. The exp->mult->OT tail is deferred 2 k-tiles
        # (the pending deque) and OT matmuls are flushed BEFORE the next S so
        # the in-order PE queue always has dependency-ready work ahead of the
        # st-slot waits.
        pending = deque()

        def flush_pending(keep=0):
            while len(pending) > keep:
                e1A, e1B, i_, j_, ot_ = pending.popleft()
                for half, e1 in ((0, e1A), (1, e1B)):
                    for (fo, fs) in hch:
                        nc.tensor.matmul(
                            ot_[:, half * hn + fo:half * hn + fo + fs],
                            lhsT=v_s[:, j_, (d + 2) * i_:(d + 2) * i_ + d + 2],
                            rhs=e1[:, fo:fo + fs],
                            start=(j_ == 0), stop=(j_ == nt - 1),
                        )

        def emit_j(i, ot, j, q0, q64, k0, k64, keep):
            """OT flush (lagged), then S (row-tiled h2 pair) + exp + mult
            for k-tile j."""
            flush_pending(keep)
            eb_t = ebp.tile([128, n], DT.bfloat16, tag="eb")
            eb_dma = nc.sync.dma_start(out=eb_t, in_=eb[i, 128 * j:128 * j + 128, :])
            if i == 0 and j < 6 and last_in_dma is not None:
                add_dep_helper(eb_dma.ins, last_in_dma.ins, sync=True,
                               reason="dma priority")
            stA = ps_st.tile([128, hn], DT.float32, tag="st", name="stA")
            stB = ps_st.tile([128, hn], DT.float32, tag="st", name="stB")
            for (fo, fs) in hch:
                nc.tensor.matmul(
                    stA[:, fo:fo + fs],
                    lhsT=k0[:, 128 * j:128 * j + 128],
                    rhs=q0[:, fo:fo + fs],
                    start=True, stop=True,
                )
                nc.tensor.matmul(
                    stB[:, fo:fo + fs],
                    lhsT=k64[:, 128 * j:128 * j + 128],
                    rhs=q64[:, hn + fo:hn + fo + fs],
                    start=True, stop=True,
                )
            e0A = e0p.tile([128, hn], DT.bfloat16, tag="e0")
            nc.scalar.activation(e0A, stA, AF.Exp)
            e1A = e1p.tile([128, hn], DT.bfloat16, tag="e1")
            nc.vector.tensor_tensor(e1A, e0A, eb_t[:, 0:hn], AluOpType.mult)
            e0B = e0p.tile([128, hn], DT.bfloat16, tag="e0")
            nc.scalar.activation(e0B, stB, AF.Exp)
            e1B = e1p.tile([128, hn], DT.bfloat16, tag="e1")
            nc.vector.tensor_tensor(e1B, e0B, eb_t[:, hn:n], AluOpType.mult)
            pending.append((e1A, e1B, i, j, ot))

        ps_ot_holder = {}

        def get_ot():
            if "pool" not in ps_ot_holder:
                ps_ot_holder["pool"] = tc.alloc_tile_pool(
                    name="ps_ot", bufs=1, space="PSUM")
            return ps_ot_holder["pool"].tile([66, n], DT.float32, tag="ot",
                                             name="ot")

        # -- head 0, early section: S/exp/mult only (OT backlog builds while
        # phase C and the B tail are emitted; the ot PSUM tile can only exist
        # after ps_qkv releases banks 4-5, so the backlog carries ot=None) --
        q0, q64, k0, k64 = head_aps(0)
        early = early_n
        c_done = 0
        for j in range(early):
            emit_j(0, None, j, q0, q64, k0, k64, keep=len(pending) + 1)
            # phase C rides along so the PE stays fed between S pairs
            for _ in range(2):
                if c_done < nt:
                    emit_c_chunk(c_done)
                    c_done += 1
        for m, ci in [(1, ci) for ci in range(kneed, nq)]:
            emit_qk_fo(m, ci)
        if ecov < n:
            nc.sync.dma_start(out=qk_d[64:128, 1, ecov:n],
                              in_=qk_s[0:64, 1, ecov:n])
        # lower-half duplicates for head 1
        nc.sync.dma_start(out=qk_d[0:64, 0, :], in_=qk_s[64:128, 0, :])
        nc.sync.dma_start(out=qk_d[0:64, 1, :], in_=qk_s[64:128, 1, :])
        while c_done < nt:
            emit_c_chunk(c_done)
            c_done += 1
        emit_qk_chunk(2)
        # head 2's swapped duplicate: qk_d[:,2] = [k2|q2]
        nc.sync.dma_start(out=qk_d[64:128, 2, :], in_=qk_s[0:64, 2, :])
        nc.sync.dma_start(out=qk_d[0:64, 2, :], in_=qk_s[64:128, 2, :])

        loadp.release()
        ps_qkv.release()

        ot = get_ot()
        # patch the backlog's ot (deferred entries carry ot=None)
        pending_fixed = deque((a, b, i_, j_, ot) for (a, b, i_, j_, _)
                              in pending)
        pending.clear()
        pending.extend(pending_fixed)

        for i in range(hpc):
            q0, q64, k0, k64 = head_aps(i)
            if i > 0:
                ot = get_ot()
            start_j = early if i == 0 else 0
            for j in range(start_j, nt):
                emit_j(i, ot, j, q0, q64, k0, k64, keep=2)
                # heads 0+1's packed normalization rides inside head 2's loop
                # (it needs both rrow0 and rrow1)
                if i == 2:
                    if j == 2:
                        emit_norm_bcast01(0)
                    elif j == 4:
                        emit_norm_bcast01(1)
                    elif j == 6:
                        emit_norm_mult01()
            flush_pending(0)
            emit_norm_head(i, ot)

        # tail: head 2's normalization
        for h2 in range(n // hn):
            emit_norm_bcast2(h2)
        emit_norm_mult2()

        ps_ot_holder["pool"].release()
        ps_st.release()

        # ---- phase F: ytT = pw01.T @ osum01 + pw2.T @ osum2 ----
        ps_pj = tc.alloc_tile_pool(name="ps_pj", bufs=2, space="PSUM")
        ytp = tc.alloc_tile_pool(name="ytp", bufs=2)
        for m in range(mo):
            ps = ps_pj.tile([128, n], DT.float32, tag="pj")
            for (fo, fs) in qch:
                nc.tensor.matmul(
                    ps[:, fo:fo + fs],
                    lhsT=pw01[:, 128 * m:128 * m + 128],
                    rhs=osum01[:, fo:fo + fs],
                    start=True, stop=False,
                )
            for (fo, fs) in qch:
                nc.tensor.matmul(
                    ps[:, fo:fo + fs],
                    lhsT=pw2[:, 128 * m:128 * m + 128],
                    rhs=osum2[:, fo:fo + fs],
                    start=False, stop=True,
                )
            yts = ytp.tile([128, n], DT.float32, tag="yts")
            nc.vector.tensor_copy(yts, ps)
            nc.sync.dma_start(out=yt[128 * m:128 * m + 128, :], in_=yts)

        ps_pj.release()
        ytp.release()
        normp.release()
        e1p.release()
        e0p.release()
        ebp.release()
        qkvout.release()
        persist.release()

    nc.compile()
    return nc


_PROG = {}


def _get_program(**kw):
    key = tuple(sorted(kw.items()))
    if key not in _PROG:
        _PROG[key] = build_program(**kw)
    return _PROG[key]


def make_in_maps(x, mask, qkv_w, qkv_b, rel_bias, proj_w):
    """Host-side shard + layout prep. Returns list of per-core input dicts."""
    x = np.asarray(x, dtype=np.float32)
    mask = np.asarray(mask)
    qkv_w = np.asarray(qkv_w, dtype=np.float32)
    qkv_b = np.asarray(qkv_b, dtype=np.float32)
    rel_bias = np.asarray(rel_bias, dtype=np.float32)
    proj_w = np.asarray(proj_w, dtype=np.float32)

    n_qk_chunks = 3
    wqk_cols = 128 * n_qk_chunks
    wv_cols = HPC * (D + 2)
    has_bias = bool(np.any(qkv_b))
    c_in = C + 1 if has_bias else C

    xts = []
    for b in range(B):
        xb = x[b].T  # [C, N]
        if has_bias:
            xb = np.concatenate([xb, np.ones((1, N), np.float32)], axis=0)
        xts.append(np.ascontiguousarray(xb))

    maps = []
    for core in range(NCORES):
        b = core // 4
        heads = [HPC * (core % 4) + i for i in range(HPC)]

        wqk = np.zeros((c_in, wqk_cols), np.float32)
        wv = np.zeros((c_in, wv_cols), np.float32)
        pwm = np.zeros((HPC * D, C), np.float32)
        for i, h in enumerate(heads):
            qw = qkv_w[D * h:D * h + D, :].T * SCALE
            kw = qkv_w[C + D * h:C + D * h + D, :].T
            if i < 2:
                base = 64 * i          # m0 = [q0|q1]
                kbase = 128 + 64 * i   # m1 = [k0|k1]
                wqk[:C, base:base + 64] = qw
                wqk[:C, kbase:kbase + 64] = kw
            else:
                # m2 = [q2|k2] (the swapped duplicate is made on-device)
                wqk[:C, 256:256 + 64] = qw
                wqk[:C, 320:320 + 64] = kw
            wv[:C, (D + 2) * i:(D + 2) * i + D] = qkv_w[2 * C + D * h:2 * C + D * h + D, :].T
            if has_bias:
                qb = qkv_b[D * h:D * h + D] * SCALE
                kb = qkv_b[C + D * h:C + D * h + D]
                if i < 2:
                    wqk[C, 64 * i:64 * i + 64] = qb
                    wqk[C, 128 + 64 * i:128 + 64 * i + 64] = kb
                else:
                    wqk[C, 256:256 + 64] = qb
                    wqk[C, 320:320 + 64] = kb
                wv[C, (D + 2) * i:(D + 2) * i + D] = qkv_b[2 * C + D * h:2 * C + D * h + D]
            pwm[64 * i:64 * i + 64, :] = proj_w[:, D * h:D * h + D].T

        ebs = np.empty((HPC, N, N), ml_dtypes.bfloat16)
        mb = (mask[b, 0] != 0)
        for i, h in enumerate(heads):
            ebs[i] = (np.exp(rel_bias[h]) * mb).T.astype(ml_dtypes.bfloat16)

        maps.append({
            "xt": xts[b].astype(ml_dtypes.bfloat16),
            "wqk": wqk.astype(ml_dtypes.bfloat16),
            "wv": wv.astype(ml_dtypes.bfloat16),
            "eb": ebs,
            "pw": pwm.astype(ml_dtypes.bfloat16),
        })
    return maps, has_bias


def kernel(x, mask, qkv_w, qkv_b, rel_bias, proj_w, proj_b):
    global LAST_RESULTS
    maps, has_bias = make_in_maps(x, mask, qkv_w, qkv_b, rel_bias, proj_w)
    nc = _get_program(c_in=C + 1 if has_bias else C)

    trace = bool(os.environ.get("KERNEL_TRACE"))
    try:
        res = run_bass_kernel_spmd(
            nc, maps, list(range(NCORES)),
            trace=trace,
            trace_cores=list(range(NCORES)) if trace else None,
        )
    except Exception:
        if not trace:
            raise
        os.environ["BASS_NEVER_TRACE"] = "1"
        res = run_bass_kernel_spmd(nc, maps, list(range(NCORES)), trace=False)
    LAST_RESULTS = res

    proj_b = np.asarray(proj_b, dtype=np.float32)
    out = np.empty((B, N, C), np.float32)
    for b in range(B):
        acc = res.results[4 * b]["yt"].astype(np.float32)
        for c in range(4 * b + 1, 4 * b + 4):
            acc = acc + res.results[c]["yt"]
        out[b] = acc.T + proj_b[None, :]
    return out
